# revision 21
# baseline (speedup 1.0000x reference)
"""Trainium2 Bass kernel for nn_DifferentiableSimulator.

Strategy (8 NeuronCores, B=8): one batch element per core, no collectives.

Host side (cheap, O(V+N)):
  - per-batch probe geometry: rotation, LUT bilinear interp (tiny)
  - per-batch voxel relevance sharding: keep voxels within CUT(8mm) +
    probe-radius of the shank axis segment (dropped voxels are suppressed
    by >= e^-9.6 relative to any weight that can influence a visible
    pixel; empirically the output matches the dense reference to ~2e-4).
    Keeps ~850 of 10k voxels -> 7 chunks of 128.
  - lattice factorization: the 1000 contacts are a rigid 10x10x10 grid,
    so in the rotated frame the soft-match weight matrix factorizes as
    W[n,v] = Wxy[(ij),v] * Wz[k,v]: 138 gaussian columns per voxel
    instead of 1000.  Voxel features ship as fp16 hi/lo pairs so the
    fp16 matmul is ~fp32-exact.
  - contacts are reindexed m = k*128 + (iy*10+ix) (28 dummy xy slots per
    z-layer, weight 0) so weighted sums land in contact-chunk layout.
  - In this input regime phos_size == 1 exactly (m_inv <= 0.82 =>
    sigma*SE <= 0.45 < 1 for any ecc in [0,12], which the weighted mean
    guarantees), so the splat gaussian args are just (i - center)^2 and
    the whole sigma-reciprocal chain drops out.

Device side (per core), phase 1 -- soft PRF match per 128-voxel chunk:
  one K=17 fp16 matmul -> xy/z gaussian exponents [128v, 138] in PSUM;
  ACT exp; one DVE op forms WzE = Wz x [pol, ecc, 1]; one fp32 matmul
  accumulates B[128ij, 30] = all weighted sums, contact-major.

Params: pol/ecc/wsum -> sin/cos via a factored-root polynomial evaluated
  in 6 fused DVE ops (no ACT Sin => the Exp table is loaded once at
  startup and NEVER reloaded).  Splat centers are split hi/lo into a
  [128, 50] coeff matrix, one PE transpose -> [50,128] fp16 lhsT.

Phase 2 -- separable splat, per z-layer chunk c:
  one k=5 fp16 matmul (coeffs x host basis [i|i],[1|0],...) emits both
  row/col args (i+center) for 256+256 pixels into PSUM; GpSimd squares
  them (x0.25 to stay in f16 range); one ACT Exp (scale -4) -> f16
  factors; DVE folds the electrode weight into the row factor; two fp16
  matmuls accumulate the 256x256 map in one [128,512] PSUM bank.
  Engine balance: PE ~0.6us/chunk, ACT ~0.6, GpSimd ~0.6, DVE ~0.3.

Max via row-reduce + GpSimd partition_all_reduce; scale on DVE+GpSimd;
two output DMAs.  A PE warmup burst at startup plus a short insurance
burst during the params window keep the HAM clock gate at 2.4 GHz.
"""
import math
from contextlib import ExitStack

import numpy as np

import concourse.bass as bass
import concourse.mybir as mybir
from concourse import bass_isa
from concourse import tile
from concourse.bass_utils import run_bass_kernel_spmd

# ---- constants (must match the reference) ----
_DEG2RAD = math.pi / 180.0
MAP_SIZE = 256
SOFT_MATCH_SIGMA = 1.5

B = 8
NCC = 10                  # contact chunks = z-layers
NXY = 128                 # xy-lattice slots per layer (100 real + 28 dummy)
CUT = 8.0
XY_RAD = 1.8 * math.sqrt(2.0)
SE = MAP_SIZE / 90.0
EXP_SCALE = 2.0 / (2.0 * SOFT_MATCH_SIGMA ** 2)   # 2/4.5
SCALE_EXP2 = -4.0 / (1.0 + 1e-8)                  # undoes the 0.25 in sq

# sin(y) = y*c4*(y^2-R1)*(y^2-R2)*((y^2+QB)*y^2+QC) on [-pi,pi]; err 2e-5
C4 = 2.17323611e-06
R1 = 9.869712469760742
R2 = 29.379373192742293
QB = -49.63325349575811
QC = 1586.862759630811

f32 = mybir.dt.float32
f16 = mybir.dt.float16
AF = mybir.ActivationFunctionType
ALU = mybir.AluOpType
PI = math.pi


# ---------------------------------------------------------------- host prep
def _f16s(x):
    hi = np.float16(x)
    lo = np.float16(np.float32(x) - np.float32(hi))
    return hi, lo


def _f16_split(x):
    hi = x.astype(np.float16)
    lo = (x.astype(np.float32) - hi.astype(np.float32)).astype(np.float16)
    return hi.astype(np.float32), lo.astype(np.float32)


def _host_geometry(params, start_loc, surf_dist_lut, alpha_grid, beta_grid):
    params = params.astype(np.float64)
    alpha, beta, offset, shank = (params[:, 0], params[:, 1],
                                  params[:, 2], params[:, 3])
    a = alpha * _DEG2RAD
    b = beta * _DEG2RAD
    ca, sa = np.cos(a), np.sin(a)
    cb, sb = np.cos(b), np.sin(b)
    Bn = params.shape[0]
    Rx = np.zeros((Bn, 3, 3)); Ry = np.zeros((Bn, 3, 3))
    Rx[:, 0, 0] = 1; Rx[:, 1, 1] = ca; Rx[:, 1, 2] = -sa
    Rx[:, 2, 1] = sa; Rx[:, 2, 2] = ca
    Ry[:, 0, 0] = cb; Ry[:, 0, 2] = sb; Ry[:, 1, 1] = 1
    Ry[:, 2, 0] = -sb; Ry[:, 2, 2] = cb
    R = Rx @ Ry
    direction = np.einsum('bij,j->bi', R, np.array([0.0, 0.0, -1.0]))
    direction = direction / np.linalg.norm(direction, axis=-1, keepdims=True)
    lut = surf_dist_lut.astype(np.float64)
    na, nb = lut.shape
    ag, bg = alpha_grid.astype(np.float64), beta_grid.astype(np.float64)
    a_norm = 2.0 * (alpha - ag[0]) / (ag[-1] - ag[0] + 1e-08) - 1.0
    b_norm = 2.0 * (beta - bg[0]) / (bg[-1] - bg[0] + 1e-08) - 1.0
    ai = np.clip((a_norm + 1.0) * 0.5 * (na - 1), 0.0, na - 1.0)
    bi = np.clip((b_norm + 1.0) * 0.5 * (nb - 1), 0.0, nb - 1.0)
    a0 = np.clip(np.floor(ai), 0, na - 1).astype(np.int64)
    b0 = np.clip(np.floor(bi), 0, nb - 1).astype(np.int64)
    a1 = np.minimum(a0 + 1, na - 1)
    b1 = np.minimum(b0 + 1, nb - 1)
    fa = ai - a0
    fb = bi - b0
    v00 = lut[a0, b0]; v01 = lut[a0, b1]; v10 = lut[a1, b0]; v11 = lut[a1, b1]
    surf = (v00 * (1 - fa) * (1 - fb) + v01 * (1 - fa) * fb
            + v10 * fa * (1 - fb) + v11 * fa * fb)
    surf = np.maximum(surf, 1.0)
    penetration = surf - shank / 2.0 - offset
    grid_center = (start_loc.astype(np.float64)[None, :]
                   + direction * penetration[:, None])
    return grid_center, R, direction, shank


def _voxel_keep(v1_pos, grid_center, axis_dir, half_len):
    d = v1_pos.astype(np.float64) - grid_center[None, :]
    t = np.clip(d @ axis_dir, -half_len, half_len)
    dist = np.linalg.norm(d - t[:, None] * axis_dir[None, :], axis=1)
    return dist <= (CUT + XY_RAD + 0.5)


def _basis50():
    """[50, 512*NCC] f16 block-diagonal splat basis.  Chunk c's rhs is the
    [50, 512] column block; only rows 5c..5c+5 are nonzero there, pairing
    with lhsT rows [ones, sby_hi, sby_lo, sbx_hi, sbx_lo] of chunk c
    (the k=50 contraction zeroes out every other chunk's coeffs)."""
    i = np.arange(MAP_SIZE, dtype=np.float32)
    z = np.zeros(MAP_SIZE, np.float32)
    o = np.ones(MAP_SIZE, np.float32)
    blk = np.stack([np.concatenate([i, i]),
                    np.concatenate([o, z]), np.concatenate([o, z]),
                    np.concatenate([z, o]), np.concatenate([z, o])])
    bas = np.zeros((5 * NCC, 512 * NCC), np.float32)
    for c in range(NCC):
        bas[5 * c:5 * (c + 1), 512 * c:512 * (c + 1)] = blk
    return bas.astype(np.float16)


def _prep_core(gc_b, R_b, shank_b, logits_b, v1_pos_k, v1_prf_k, VP):
    """Per-core device input arrays (packed into two DMA blocks)."""
    Vk = v1_pos_k.shape[0]
    nch = VP // 128
    w = np.zeros((VP, 3))
    w[:Vk] = (v1_pos_k.astype(np.float64) - gc_b[None, :]) @ R_b
    wf = w.astype(np.float32)
    wh, wl = _f16_split(wf)
    bxy = (-0.5 * (w[:, 0] ** 2 + w[:, 1] ** 2)).astype(np.float32)
    bz = (-0.5 * w[:, 2] ** 2).astype(np.float32)
    bxy[Vk:] = -30000.0
    bz[Vk:] = -30000.0
    bxyh, bxyl = _f16_split(bxy)
    bzh, bzl = _f16_split(bz)
    onesv = np.ones(VP, np.float32)
    vt = np.stack([wh[:, 0], wh[:, 1], wl[:, 0], wl[:, 1], wh[:, 0],
                   wh[:, 1], onesv, onesv, bxyh, bxyl,
                   wh[:, 2], wl[:, 2], wh[:, 2], onesv, onesv, bzh, bzl],
                  axis=0).astype(np.float16)

    xs = np.arange(10) * 0.4 - 1.8
    zs = (np.linspace(0.0, 1.0, 10) - 0.5) * float(shank_b)
    cols = np.zeros((17, NXY + 10), np.float32)
    for ij in range(NXY):
        if ij < 100:
            iy, ix = ij // 10, ij % 10
            x, y = xs[ix], xs[iy]
            xh, xl = _f16s(x)
            yh, yl = _f16s(y)
            axyh, axyl = _f16s(-0.5 * (x * x + y * y))
            cols[0:10, ij] = [xh, yh, xh, yh, xl, yl, axyh, axyl, 1.0, 1.0]
        else:
            cols[6, ij] = -30000.0     # dummy xy slot -> Wxy = 0
            cols[8, ij] = 1.0
    for k in range(10):
        z = zs[k]
        zh, zl = _f16s(z)
        azh, azl = _f16s(-0.5 * z * z)
        cols[10:17, NXY + k] = [zh, zh, zl, azh, azl, 1.0, 1.0]
    rhs = cols.astype(np.float16)

    d16 = np.concatenate([vt, rhs], axis=1)        # [17, VP+138]

    e3 = np.zeros((VP, 3), np.float32)
    e3[:Vk, 0] = v1_prf_k[:, 0]
    e3[:Vk, 1] = v1_prf_k[:, 1]
    e3[:Vk, 2] = 1.0
    e3t = np.ascontiguousarray(
        e3.reshape(nch, 128, 3).transpose(1, 0, 2).reshape(128, 3 * nch))

    lgt = np.full((NXY, NCC), -30.0, np.float32)
    iy, ix = np.divmod(np.arange(100), 10)
    for k in range(NCC):
        lgt[:100, k] = logits_b[iy * 100 + ix * 10 + k]
    d32 = np.concatenate([e3t, lgt, np.eye(128, dtype=np.float32)], axis=1)
    return {"d16": np.ascontiguousarray(d16),
            "d32": np.ascontiguousarray(d32),
            "bas": _basis50()}


# ------------------------------------------------------------- device kernel
def _split_multiwaits(nc):
    """This walrus build accepts at most ONE sync wait per instruction.
    Tile emits several.  Engine instruction streams execute in order, so
    moving all but one wait onto single-wait NoOps inserted just before
    the instruction preserves semantics exactly."""
    cnt = 0
    for fn in nc.m.functions:
        for blk in fn.blocks:
            out = []
            for inst in blk.instructions:
                si = inst.sync_info
                if si is not None and si.on_wait is not None \
                        and len(si.on_wait) > 1:
                    waits = list(si.on_wait)
                    for w in waits[:-1]:
                        cnt += 1
                        out.append(mybir.InstNoOp(
                            name=f"WSPLIT-{cnt}",
                            engine=inst.engine,
                            ins=[], outs=[],
                            sync_info=mybir.SyncInfo(on_wait=[w],
                                                     on_update=[]),
                        ))
                    inst.sync_info = mybir.SyncInfo(
                        on_wait=[waits[-1]], on_update=list(si.on_update))
                out.append(inst)
            blk.instructions = out
    return cnt


def _build_nc(VP):
    nch = VP // 128
    NL = NXY + 10          # 138 lattice columns
    W16 = VP + NL
    W32 = 3 * nch + NCC + 128
    nc = bass.Bass()
    d16_d = nc.dram_tensor("d16", [17, W16], f16, kind="ExternalInput")
    d32_d = nc.dram_tensor("d32", [128, W32], f32, kind="ExternalInput")
    bas_d = nc.dram_tensor("bas", [5 * NCC, 512 * NCC], f16,
                           kind="ExternalInput")
    out_d = nc.dram_tensor("out", [MAP_SIZE, MAP_SIZE], f32,
                           kind="ExternalOutput")

    with ExitStack() as ctx:
        tc = ctx.enter_context(tile.TileContext(nc))
        constp = ctx.enter_context(tc.tile_pool(name="const", bufs=1))
        parm = ctx.enter_context(tc.tile_pool(name="parm", bufs=1))
        work = ctx.enter_context(tc.tile_pool(name="work", bufs=6))
        psA = ctx.enter_context(
            tc.tile_pool(name="psA", bufs=1, space=bass.MemorySpace.PSUM))
        psB = ctx.enter_context(
            tc.tile_pool(name="psB", bufs=1, space=bass.MemorySpace.PSUM))
        psM = ctx.enter_context(
            tc.tile_pool(name="psM", bufs=1, space=bass.MemorySpace.PSUM))
        psY = ctx.enter_context(
            tc.tile_pool(name="psY", bufs=2, space=bass.MemorySpace.PSUM))

        # Warmups first (top scheduler priority): ACT exp-table load + PE
        # HAM burst run during the sem-init + input-DMA window.
        scr = constp.tile([1, 1], f32, tag="scr", name="scr")
        nc.vector.memset(scr[:], 0.0)
        nc.scalar.activation(scr[:], scr[:], AF.Exp, bias=0.0, scale=1.0)
        wrm = constp.tile([128, 512], f16, tag="wrm", name="wrm")
        nc.vector.memset(wrm[:], 0.0)
        wps = psA.tile([128, 512], f32, tag="wps", name="wps")
        for _ in range(12):
            nc.tensor.matmul(wps[:], wrm[:, 0:128], wrm[:],
                             start=True, stop=True, skip_group_check=True)

        dp16 = constp.tile([17, W16], f16, tag="dp16", name="dp16")
        nc.sync.dma_start(dp16[:], d16_d[:])
        dp32 = constp.tile([128, W32], f32, tag="dp32", name="dp32")
        nc.scalar.dma_start(dp32[:], d32_d[:])
        bas_t = constp.tile([5 * NCC, 512 * NCC], f16, tag="bas", name="bas")
        nc.gpsimd.dma_start(bas_t[:], bas_d[:])
        vt_t = dp16[:, 0:VP]
        rhs_t = dp16[:, VP:VP + NL]
        e3_t = dp32[:, 0:3 * nch]
        lg_t = dp32[:, 3 * nch:3 * nch + NCC]
        eye_t = dp32[:, 3 * nch + NCC:W32]

        ones_t = constp.tile([1, 128], f32, tag="ones", name="ones")
        nc.vector.memset(ones_t[:], 1.0)
        konst = constp.tile([128, 2 * NCC], f32, tag="konst", name="konst")
        nc.vector.memset(konst[:, 0:NCC], -128.0)
        nc.vector.memset(konst[:, NCC:2 * NCC], -127.0)
        ctile = constp.tile([128, 5 * NCC], f32, tag="ctile", name="ctile")
        cj = ctile[:].rearrange("p (c j) -> p c j", j=5)
        nc.vector.memset(cj[:, :, 0], 1.0)

        # sigmoid(logits): independent of phase 1, runs early.
        en = parm.tile([128, NCC], f32, tag="en", name="en")
        nc.scalar.activation(en[:], lg_t, AF.Exp, bias=0.0, scale=-1.0)
        nc.vector.tensor_scalar_add(en[:], en[:], 1.0)
        pb = parm.tile([128, NCC], f32, tag="pb", name="pb")
        nc.vector.reciprocal(pb[:], en[:])

        # ---------------- phase 1: factorized soft match ----------------
        B_ps = psB.tile([128, 3 * NCC], f32, tag="B", name="B")
        with tc.tile_pool(name="psW", bufs=2,
                          space=bass.MemorySpace.PSUM) as psW:
            for k in range(nch):
                ct = psW.tile([128, NL], f32, tag="cross", name="cross")
                nc.tensor.matmul(ct[:], vt_t[:, k * 128:(k + 1) * 128],
                                 rhs_t, start=True, stop=True)
                wx = work.tile([128, NL], f32, tag="wx", name="wx")
                nc.scalar.activation(wx[:], ct[:], AF.Exp,
                                     bias=0.0, scale=EXP_SCALE)
                wze = work.tile([128, 3 * NCC], f32, tag="wze", name="wze")
                e3b = e3_t[:, 3 * k:3 * k + 3] \
                    .rearrange("p (one f) -> p one f", one=1) \
                    .broadcast_to([128, NCC, 3])
                wzb = wx[:, NXY:NL] \
                    .rearrange("p (k one) -> p k one", one=1) \
                    .broadcast_to([128, NCC, 3])
                nc.vector.tensor_tensor(
                    wze[:].rearrange("p (k f) -> p k f", f=3),
                    e3b, wzb, ALU.mult)
                nc.tensor.matmul(B_ps[:], wx[:, 0:NXY], wze[:],
                                 start=(k == 0), stop=(k == nch - 1))

        bsb = parm.tile([128, 3 * NCC], f32, tag="bsb", name="bsb")
        nc.vector.tensor_copy(bsb[:], B_ps[:])
        bs3 = bsb[:].rearrange("p (k f) -> p k f", f=3)

        def pt(tag, w=NCC):
            return parm.tile([128, w], f32, tag=tag, name=tag)

        # ---------------- per-contact params ----------------
        t0 = pt("t0")
        nc.vector.tensor_scalar_add(t0[:], bs3[:, :, 2], 1e-8)
        rws = pt("rws"); nc.vector.reciprocal(rws[:], t0[:])
        pol = pt("pol")
        nc.vector.tensor_mul(pol[:], bs3[:, :, 0], rws[:])
        ecc = pt("ecc")
        nc.vector.tensor_mul(ecc[:], bs3[:, :, 1], rws[:])

        # t20 = [t | |t| - pi/2], t = pol*rad - pi;  poly gives
        # [sin(t) | -cos(t)]  (factored-root form, 6 fused DVE ops).
        t20 = pt("t20", 2 * NCC)
        nc.vector.tensor_scalar(t20[:, 0:NCC], pol[:], _DEG2RAD, -PI,
                                ALU.mult, ALU.add)
        nc.vector.scalar_tensor_tensor(t20[:, NCC:2 * NCC], t20[:, 0:NCC],
                                       -1.0, t20[:, 0:NCC],
                                       ALU.mult, ALU.max)
        nc.vector.tensor_scalar_add(t20[:, NCC:2 * NCC],
                                    t20[:, NCC:2 * NCC], -PI / 2.0)
        px = pt("px", 2 * NCC)
        nc.vector.tensor_mul(px[:], t20[:], t20[:])
        pu = pt("pu", 2 * NCC)
        nc.vector.tensor_scalar_mul(pu[:], t20[:], C4)
        pa = pt("pa", 2 * NCC)
        nc.vector.scalar_tensor_tensor(pa[:], px[:], -R1, pu[:],
                                       ALU.add, ALU.mult)
        pb2 = pt("pb2", 2 * NCC)
        nc.vector.scalar_tensor_tensor(pb2[:], px[:], -R2, pa[:],
                                       ALU.add, ALU.mult)
        pq = pt("pq", 2 * NCC)
        nc.vector.scalar_tensor_tensor(pq[:], px[:], QB, px[:],
                                       ALU.add, ALU.mult)
        sc20 = pt("sc20", 2 * NCC)
        nc.vector.scalar_tensor_tensor(sc20[:], pq[:], QC, pb2[:],
                                       ALU.add, ALU.mult)

        # t12 = ecc * [sn | -cs];  nxy = t12*SE + [-128 | -127]
        t12 = pt("t12", 2 * NCC)
        eb = ecc[:].rearrange("p (one k) -> p one k", one=1) \
            .broadcast_to([128, 2, NCC])
        nc.vector.tensor_tensor(t12[:].rearrange("p (two k) -> p two k",
                                                 two=2),
                                sc20[:].rearrange("p (two k) -> p two k",
                                                  two=2),
                                eb, ALU.mult)
        nxy = pt("nxy", 2 * NCC)
        nc.vector.scalar_tensor_tensor(nxy[:], t12[:], SE, konst[:],
                                       ALU.mult, ALU.add)

        # hi/lo split -> [128, 50] coeff matrix -> PE transpose -> fp16 lhsT
        hi16 = parm.tile([128, 2 * NCC], f16, tag="hi16", name="hi16")
        nc.vector.tensor_copy(hi16[:], nxy[:])
        nc.vector.tensor_copy(cj[:, :, 1], hi16[:, NCC:2 * NCC])
        nc.vector.tensor_copy(cj[:, :, 3], hi16[:, 0:NCC])
        nc.vector.tensor_sub(cj[:, :, 2], nxy[:, NCC:2 * NCC],
                             hi16[:, NCC:2 * NCC])
        nc.vector.tensor_sub(cj[:, :, 4], nxy[:, 0:NCC], hi16[:, 0:NCC])
        lhsT = parm.tile([5 * NCC, 128], f16, tag="lhsT", name="lhsT")
        with tc.tile_pool(name="psT", bufs=1,
                          space=bass.MemorySpace.PSUM) as psT:
            coefT = psT.tile([5 * NCC, 128], f32, tag="coefT", name="coefT")
            nc.tensor.transpose(coefT[:], ctile[:], eye_t)
            nc.vector.tensor_copy(lhsT[:], coefT[:])

        val = pt("val")
        nc.vector.tensor_scalar_min(val[:], bs3[:, :, 2], 1.0)
        wc = pt("wc"); nc.vector.tensor_mul(wc[:], pb[:], val[:])

        # HAM insurance: keep the PE active through the params window so
        # phase 2 still runs at the fast clock.
        for _ in range(4):
            nc.tensor.matmul(wps[:], wrm[:, 0:128], wrm[:],
                             start=True, stop=True, skip_group_check=True)

        # ---------------- phase 2: separable splat ----------------
        mp = psM.tile([128, 2 * MAP_SIZE], f32, tag="map", name="map")

        def consume(c, ys):
            # DVE drains PSUM->f16 with exact /2 (one PSUM operand max);
            # GpSimd squares from SBUF -> sq = ys^2/4 stays in f16 range.
            ysb = work.tile([128, 2 * MAP_SIZE], f16, tag="ysb", name="ysb")
            nc.vector.tensor_scalar_mul(ysb[:], ys[:], 0.5)
            sq = work.tile([128, 2 * MAP_SIZE], f16, tag="sq", name="sq")
            nc.gpsimd.tensor_mul(sq[:], ysb[:], ysb[:])
            xy8 = work.tile([128, 2 * MAP_SIZE], f16, tag="xy8", name="xy8")
            nc.scalar.activation(xy8[:], sq[:], AF.Exp,
                                 bias=0.0, scale=SCALE_EXP2)
            yy = work.tile([128, MAP_SIZE], f16, tag="yy", name="yy")
            nc.vector.tensor_scalar_mul(yy[:], xy8[:, 0:MAP_SIZE],
                                        wc[:, c:c + 1])
            xx = xy8[:, MAP_SIZE:2 * MAP_SIZE]
            nc.tensor.matmul(mp[:, 0:MAP_SIZE], yy[:, 0:128], xx,
                             start=(c == 0), stop=(c == NCC - 1))
            nc.tensor.matmul(mp[:, MAP_SIZE:2 * MAP_SIZE], yy[:, 128:256],
                             xx, start=(c == 0), stop=(c == NCC - 1))

        prev = None
        for c in range(NCC):
            ys = psY.tile([128, 2 * MAP_SIZE], f32, tag="ys", name="ys")
            nc.tensor.matmul(ys[:], lhsT[:],
                             bas_t[:, 512 * c:512 * (c + 1)],
                             start=True, stop=True)
            if prev is not None:
                consume(*prev)
            prev = (c, ys)
        consume(*prev)

        # ---------------- normalize + store ----------------
        mx = parm.tile([128, 1], f32, tag="mx", name="mx")
        nc.vector.reduce_max(mx[:], mp[:], axis=mybir.AxisListType.X)
        with tc.tile_pool(name="psG", bufs=1,
                          space=bass.MemorySpace.PSUM) as psG:
            mt = psG.tile([1, 128], f32, tag="mt", name="mt")
            nc.tensor.transpose(mt[:], mx[:], eye_t)
            gm = parm.tile([1, 1], f32, tag="gm", name="gm")
            nc.vector.reduce_max(gm[:], mt[:], axis=mybir.AxisListType.X)
            nc.vector.tensor_scalar_add(gm[:], gm[:], 1e-8)
            gi = parm.tile([1, 1], f32, tag="gi", name="gi")
            nc.vector.reciprocal(gi[:], gm[:])
            gb = psG.tile([128, 1], f32, tag="gb", name="gb")
            nc.tensor.matmul(gb[:], ones_t[:], gi[:], start=True, stop=True)
            gs = parm.tile([128, 1], f32, tag="gs", name="gs")
            nc.vector.tensor_copy(gs[:], gb[:])

        o0 = work.tile([128, MAP_SIZE], f32, tag="o0", name="o0")
        nc.vector.tensor_scalar_mul(o0[:], mp[:, 0:MAP_SIZE], gs[:])
        o1 = work.tile([128, MAP_SIZE], f32, tag="o1", name="o1")
        nc.scalar.activation(o1[:], mp[:, MAP_SIZE:2 * MAP_SIZE],
                             AF.Copy, scale=gs[:])
        nc.sync.dma_start(out_d[0:128, :], o0[:])
        nc.scalar.dma_start(out_d[128:256, :], o1[:])
    return nc


# ----------------------------------------------------------------- entry
def _run(inputs, trace=False):
    params = np.asarray(inputs["params"], np.float32)
    logits = np.asarray(inputs["electrode_logits"], np.float32)
    v1_pos = np.asarray(inputs["v1_pos"], np.float32)
    v1_prf = np.asarray(inputs["v1_prf"], np.float32)
    start_loc = np.asarray(inputs["start_loc"], np.float32)
    surf_dist_lut = np.asarray(inputs["surf_dist_lut"], np.float32)
    alpha_grid = np.asarray(inputs["alpha_grid"], np.float32)
    beta_grid = np.asarray(inputs["beta_grid"], np.float32)

    gc, R, direction, shank = _host_geometry(
        params, start_loc, surf_dist_lut, alpha_grid, beta_grid)
    keeps = [_voxel_keep(v1_pos, gc[b], R[b, :, 2], shank[b] / 2.0)
             for b in range(B)]
    nkeep = max(int(k.sum()) for k in keeps)
    VP = max(128, ((nkeep + 127) // 128) * 128)

    in_maps = []
    for b in range(B):
        k = keeps[b]
        in_maps.append(_prep_core(gc[b], R[b], shank[b], logits[b],
                                  v1_pos[k], v1_prf[k], VP))
    nc = _build_nc(VP)
    _split_multiwaits(nc)
    res = run_bass_kernel_spmd(nc, in_maps, list(range(B)), trace=trace)
    out = np.stack([res.results[i]["out"] for i in range(B)])
    return out[:, None, :, :].astype(np.float32), res


def kernel(**inputs) -> np.ndarray:
    out, _ = _run(inputs, trace=False)
    return out


# revision 29
# speedup vs baseline: 1.0846x; 1.0846x over previous
"""Trainium2 Bass kernel for nn_DifferentiableSimulator.

Strategy (8 NeuronCores, B=8): one batch element per core, no collectives.

Host side (cheap, O(V+N)):
  - per-batch probe geometry: rotation, LUT bilinear interp (tiny)
  - per-batch voxel relevance sharding: keep voxels within CUT(8mm) +
    probe-radius of the shank axis segment (dropped voxels are suppressed
    by >= e^-9.6 relative to any weight that can influence a visible
    pixel; empirically the output matches the dense reference to ~2e-4).
    Keeps ~850 of 10k voxels -> 7 chunks of 128.
  - lattice factorization: the 1000 contacts are a rigid 10x10x10 grid,
    so in the rotated frame the soft-match weight matrix factorizes as
    W[n,v] = Wxy[(ij),v] * Wz[k,v]: 138 gaussian columns per voxel
    instead of 1000.  Voxel features ship as fp16 hi/lo pairs so the
    fp16 matmul is ~fp32-exact.
  - contacts are reindexed m = k*128 + (iy*10+ix) (28 dummy xy slots per
    z-layer, weight 0) so weighted sums land in contact-chunk layout.
  - In this input regime phos_size == 1 exactly (m_inv <= 0.82 =>
    sigma*SE <= 0.45 < 1 for any ecc in [0,12], which the weighted mean
    guarantees), so the splat gaussian args are just (i - center)^2 and
    the whole sigma-reciprocal chain drops out.

Device side (per core), phase 1 -- soft PRF match per 128-voxel chunk:
  one K=17 fp16 matmul -> xy/z gaussian exponents [128v, 138] in PSUM;
  ACT exp; one DVE op forms WzE = Wz x [pol, ecc, 1]; one fp32 matmul
  accumulates B[128ij, 30] = all weighted sums, contact-major.

Params: pol/ecc/wsum -> sin/cos via a factored-root polynomial evaluated
  in 6 fused DVE ops (no ACT Sin => the Exp table is loaded once at
  startup and NEVER reloaded).  Splat centers are split hi/lo into a
  [128, 50] coeff matrix, one PE transpose -> [50,128] fp16 lhsT.

Phase 2 -- separable splat.  The SQUARED args are emitted directly by
  the PE: (i+s)^2 = i^2 + 2i*s + s^2 with i^2 shipped as exact f16
  hi/lo basis rows, s as a 3-level f16 split (coeff rows), and s^2 as
  its f16 head; the s^2 residual folds into the per-contact weight via
  one tiny exp (all bounded so no f16 overflow).  One k=100 fp16 matmul
  per chunk (block-diagonal [100, 5120] host basis, DMA'd early on an
  idle queue) -> [128, 512] squared args in PSUM; chunks are consumed in
  pairs: one ACT Exp [128,1024] straight from PSUM -> f16 factors; DVE
  folds wc*wfac into the row factor; four fp16 matmuls per pair
  accumulate the 256x256 map in one [128,512] PSUM bank.

Max via row-reduce + PE transpose; f16 reciprocal broadcast; scale on
DVE + ACT; two output DMAs.  A PE warmup burst at startup plus
data-dependent insurance matmuls (on bsb/hi16, so the scheduler cannot
hoist them) keep the HAM clock gate at 2.4 GHz through phase 2.
"""
import math
from contextlib import ExitStack

import numpy as np

import concourse.bass as bass
import concourse.mybir as mybir
from concourse import bass_isa
from concourse import tile
from concourse.bass_utils import run_bass_kernel_spmd

# ---- constants (must match the reference) ----
_DEG2RAD = math.pi / 180.0
MAP_SIZE = 256
SOFT_MATCH_SIGMA = 1.5

B = 8
NCC = 10                  # contact chunks = z-layers
NXY = 128                 # xy-lattice slots per layer (100 real + 28 dummy)
CUT = 8.0
XY_RAD = 1.8 * math.sqrt(2.0)
SE = MAP_SIZE / 90.0
EXP_SCALE = 2.0 / (2.0 * SOFT_MATCH_SIGMA ** 2)   # 2/4.5
SCALE_EXP2 = -4.0 / (1.0 + 1e-8)                  # undoes the 0.25 in sq

# sin(y) = y*c4*(y^2-R1)*(y^2-R2)*((y^2+QB)*y^2+QC) on [-pi,pi]; err 2e-5
C4 = 2.17323611e-06
R1 = 9.869712469760742
R2 = 29.379373192742293
QB = -49.63325349575811
QC = 1586.862759630811

f32 = mybir.dt.float32
f16 = mybir.dt.float16
AF = mybir.ActivationFunctionType
ALU = mybir.AluOpType
PI = math.pi


# ---------------------------------------------------------------- host prep
def _f16s(x):
    hi = np.float16(x)
    lo = np.float16(np.float32(x) - np.float32(hi))
    return hi, lo


def _f16_split(x):
    hi = x.astype(np.float16)
    lo = (x.astype(np.float32) - hi.astype(np.float32)).astype(np.float16)
    return hi.astype(np.float32), lo.astype(np.float32)


def _host_geometry(params, start_loc, surf_dist_lut, alpha_grid, beta_grid):
    params = params.astype(np.float64)
    alpha, beta, offset, shank = (params[:, 0], params[:, 1],
                                  params[:, 2], params[:, 3])
    a = alpha * _DEG2RAD
    b = beta * _DEG2RAD
    ca, sa = np.cos(a), np.sin(a)
    cb, sb = np.cos(b), np.sin(b)
    Bn = params.shape[0]
    Rx = np.zeros((Bn, 3, 3)); Ry = np.zeros((Bn, 3, 3))
    Rx[:, 0, 0] = 1; Rx[:, 1, 1] = ca; Rx[:, 1, 2] = -sa
    Rx[:, 2, 1] = sa; Rx[:, 2, 2] = ca
    Ry[:, 0, 0] = cb; Ry[:, 0, 2] = sb; Ry[:, 1, 1] = 1
    Ry[:, 2, 0] = -sb; Ry[:, 2, 2] = cb
    R = Rx @ Ry
    direction = np.einsum('bij,j->bi', R, np.array([0.0, 0.0, -1.0]))
    direction = direction / np.linalg.norm(direction, axis=-1, keepdims=True)
    lut = surf_dist_lut.astype(np.float64)
    na, nb = lut.shape
    ag, bg = alpha_grid.astype(np.float64), beta_grid.astype(np.float64)
    a_norm = 2.0 * (alpha - ag[0]) / (ag[-1] - ag[0] + 1e-08) - 1.0
    b_norm = 2.0 * (beta - bg[0]) / (bg[-1] - bg[0] + 1e-08) - 1.0
    ai = np.clip((a_norm + 1.0) * 0.5 * (na - 1), 0.0, na - 1.0)
    bi = np.clip((b_norm + 1.0) * 0.5 * (nb - 1), 0.0, nb - 1.0)
    a0 = np.clip(np.floor(ai), 0, na - 1).astype(np.int64)
    b0 = np.clip(np.floor(bi), 0, nb - 1).astype(np.int64)
    a1 = np.minimum(a0 + 1, na - 1)
    b1 = np.minimum(b0 + 1, nb - 1)
    fa = ai - a0
    fb = bi - b0
    v00 = lut[a0, b0]; v01 = lut[a0, b1]; v10 = lut[a1, b0]; v11 = lut[a1, b1]
    surf = (v00 * (1 - fa) * (1 - fb) + v01 * (1 - fa) * fb
            + v10 * fa * (1 - fb) + v11 * fa * fb)
    surf = np.maximum(surf, 1.0)
    penetration = surf - shank / 2.0 - offset
    grid_center = (start_loc.astype(np.float64)[None, :]
                   + direction * penetration[:, None])
    return grid_center, R, direction, shank


def _voxel_keep(v1_pos, grid_center, axis_dir, half_len):
    d = v1_pos.astype(np.float64) - grid_center[None, :]
    t = np.clip(d @ axis_dir, -half_len, half_len)
    dist = np.linalg.norm(d - t[:, None] * axis_dir[None, :], axis=1)
    return dist <= (CUT + XY_RAD + 0.5)


NJ = 10     # coeff rows per chunk -> k = 100


def _basis100():
    """[100, 512*NCC] f16 block-diagonal squared-args basis.  Chunk c's
    rhs is the [100, 512] column block; only rows 10c..10c+10 are nonzero
    there, pairing with lhsT coeff rows
    [1, 1, sy_h, sy_l, sy_l2, sys_h, sx_h, sx_l, sx_l2, sxs_h]:
      arg_y(i) = i2_h + i2_l + 2i*(sy_h+sy_l+sy_l2) + sys_h   (cols 0:256)
      arg_x(i) = likewise with sx/sxs                         (cols 256:512)
    i^2 = i2_h + i2_l exactly in f16; 2i <= 510 is f16-exact."""
    i = np.arange(MAP_SIZE, dtype=np.float64)
    i2 = i * i
    i2h = i2.astype(np.float16).astype(np.float64)
    i2l = i2 - i2h
    z = np.zeros(MAP_SIZE)
    o = np.ones(MAP_SIZE)
    ti = 2.0 * i
    blk = np.stack([np.concatenate([i2h, i2h]),
                    np.concatenate([i2l, i2l]),
                    np.concatenate([ti, z]), np.concatenate([ti, z]),
                    np.concatenate([ti, z]), np.concatenate([o, z]),
                    np.concatenate([z, ti]), np.concatenate([z, ti]),
                    np.concatenate([z, ti]), np.concatenate([z, o])])
    bas = np.zeros((NJ * NCC, 512 * NCC), np.float64)
    for c in range(NCC):
        bas[NJ * c:NJ * (c + 1), 512 * c:512 * (c + 1)] = blk
    return bas.astype(np.float16)


def _prep_core(gc_b, R_b, shank_b, logits_b, v1_pos_k, v1_prf_k, VP):
    """Per-core device input arrays (packed into two DMA blocks)."""
    Vk = v1_pos_k.shape[0]
    nch = VP // 128
    w = np.zeros((VP, 3))
    w[:Vk] = (v1_pos_k.astype(np.float64) - gc_b[None, :]) @ R_b
    wf = w.astype(np.float32)
    wh, wl = _f16_split(wf)
    bxy = (-0.5 * (w[:, 0] ** 2 + w[:, 1] ** 2)).astype(np.float32)
    bz = (-0.5 * w[:, 2] ** 2).astype(np.float32)
    bxy[Vk:] = -30000.0
    bz[Vk:] = -30000.0
    bxyh, bxyl = _f16_split(bxy)
    bzh, bzl = _f16_split(bz)
    onesv = np.ones(VP, np.float32)
    vt = np.stack([wh[:, 0], wh[:, 1], wl[:, 0], wl[:, 1], wh[:, 0],
                   wh[:, 1], onesv, onesv, bxyh, bxyl,
                   wh[:, 2], wl[:, 2], wh[:, 2], onesv, onesv, bzh, bzl],
                  axis=0).astype(np.float16)

    xs = np.arange(10) * 0.4 - 1.8
    zs = (np.linspace(0.0, 1.0, 10) - 0.5) * float(shank_b)
    cols = np.zeros((17, NXY + 10), np.float32)
    for ij in range(NXY):
        if ij < 100:
            iy, ix = ij // 10, ij % 10
            x, y = xs[ix], xs[iy]
            xh, xl = _f16s(x)
            yh, yl = _f16s(y)
            axyh, axyl = _f16s(-0.5 * (x * x + y * y))
            cols[0:10, ij] = [xh, yh, xh, yh, xl, yl, axyh, axyl, 1.0, 1.0]
        else:
            cols[6, ij] = -30000.0     # dummy xy slot -> Wxy = 0
            cols[8, ij] = 1.0
    for k in range(10):
        z = zs[k]
        zh, zl = _f16s(z)
        azh, azl = _f16s(-0.5 * z * z)
        cols[10:17, NXY + k] = [zh, zh, zl, azh, azl, 1.0, 1.0]
    rhs = cols.astype(np.float16)

    d16 = np.concatenate([vt, rhs], axis=1)        # [17, VP+138]

    e3 = np.zeros((VP, 3), np.float32)
    e3[:Vk, 0] = v1_prf_k[:, 0]
    e3[:Vk, 1] = v1_prf_k[:, 1]
    e3[:Vk, 2] = 1.0
    e3t = np.ascontiguousarray(
        e3.reshape(nch, 128, 3).transpose(1, 0, 2).reshape(128, 3 * nch))

    lgt = np.full((NXY, NCC), -30.0, np.float32)
    iy, ix = np.divmod(np.arange(100), 10)
    for k in range(NCC):
        lgt[:100, k] = logits_b[iy * 100 + ix * 10 + k]
    d32 = np.concatenate([e3t, lgt, np.eye(128, dtype=np.float32)], axis=1)
    return {"d16": np.ascontiguousarray(d16),
            "d32": np.ascontiguousarray(d32),
            "bas": _basis100()}


# ------------------------------------------------------------- device kernel
def _split_multiwaits(nc):
    """This walrus build accepts at most ONE sync wait per instruction.
    Tile emits several.  Engine instruction streams execute in order, so
    moving all but one wait onto single-wait NoOps inserted just before
    the instruction preserves semantics exactly."""
    cnt = 0
    for fn in nc.m.functions:
        for blk in fn.blocks:
            out = []
            for inst in blk.instructions:
                si = inst.sync_info
                if si is not None and si.on_wait is not None \
                        and len(si.on_wait) > 1:
                    waits = list(si.on_wait)
                    for w in waits[:-1]:
                        cnt += 1
                        out.append(mybir.InstNoOp(
                            name=f"WSPLIT-{cnt}",
                            engine=inst.engine,
                            ins=[], outs=[],
                            sync_info=mybir.SyncInfo(on_wait=[w],
                                                     on_update=[]),
                        ))
                    inst.sync_info = mybir.SyncInfo(
                        on_wait=[waits[-1]], on_update=list(si.on_update))
                out.append(inst)
            blk.instructions = out
    return cnt


def _build_nc(VP):
    nch = VP // 128
    NL = NXY + 10          # 138 lattice columns
    W16 = VP + NL
    W32 = 3 * nch + NCC + 128
    nc = bass.Bass()
    d16_d = nc.dram_tensor("d16", [17, W16], f16, kind="ExternalInput")
    d32_d = nc.dram_tensor("d32", [128, W32], f32, kind="ExternalInput")
    bas_d = nc.dram_tensor("bas", [NJ * NCC, 512 * NCC], f16,
                           kind="ExternalInput")
    out_d = nc.dram_tensor("out", [MAP_SIZE, MAP_SIZE], f32,
                           kind="ExternalOutput")

    with ExitStack() as ctx:
        tc = ctx.enter_context(tile.TileContext(nc))
        constp = ctx.enter_context(tc.tile_pool(name="const", bufs=1))
        parm = ctx.enter_context(tc.tile_pool(name="parm", bufs=1))
        work = ctx.enter_context(tc.tile_pool(name="work", bufs=6))
        psA = ctx.enter_context(
            tc.tile_pool(name="psA", bufs=1, space=bass.MemorySpace.PSUM))

        # Warmups first (top scheduler priority): ACT exp-table load + PE
        # HAM burst run during the sem-init + input-DMA window.
        scr = constp.tile([1, 1], f32, tag="scr", name="scr")
        nc.vector.memset(scr[:], 0.0)
        nc.scalar.activation(scr[:], scr[:], AF.Exp, bias=0.0, scale=1.0)
        wrm = constp.tile([128, 512], f16, tag="wrm", name="wrm")
        nc.vector.memset(wrm[:], 0.0)
        wps = psA.tile([128, 512], f32, tag="wps", name="wps")
        for _ in range(8):
            nc.tensor.matmul(wps[:], wrm[:, 0:128], wrm[:],
                             start=True, stop=True, skip_group_check=True)

        bas_t = constp.tile([NJ * NCC, 512 * NCC], f16, tag="bas",
                            name="bas")
        nc.gpsimd.dma_start(bas_t[:], bas_d[:])
        dp16 = constp.tile([17, W16], f16, tag="dp16", name="dp16")
        nc.sync.dma_start(dp16[:], d16_d[:])
        dp32 = constp.tile([128, W32], f32, tag="dp32", name="dp32")
        nc.scalar.dma_start(dp32[:], d32_d[:])
        vt_t = dp16[:, 0:VP]
        rhs_t = dp16[:, VP:VP + NL]
        e3_t = dp32[:, 0:3 * nch]
        lg_t = dp32[:, 3 * nch:3 * nch + NCC]
        eye_t = dp32[:, 3 * nch + NCC:W32]

        ones16 = constp.tile([1, 128], f16, tag="ones16", name="ones16")
        nc.vector.memset(ones16[:], 1.0)
        konst = constp.tile([128, 2 * NCC], f32, tag="konst", name="konst")
        nc.vector.memset(konst[:, 0:NCC], -128.0)
        nc.vector.memset(konst[:, NCC:2 * NCC], -127.0)
        ctile = constp.tile([128, NJ * NCC], f32, tag="ctile", name="ctile")
        cj = ctile[:].rearrange("p (c j) -> p c j", j=NJ)
        nc.vector.memset(cj[:, :, 0], 1.0)
        nc.vector.memset(cj[:, :, 1], 1.0)

        # sigmoid(logits): independent of phase 1, runs early.
        en = parm.tile([128, NCC], f32, tag="en", name="en")
        nc.scalar.activation(en[:], lg_t, AF.Exp, bias=0.0, scale=-1.0)
        nc.vector.tensor_scalar_add(en[:], en[:], 1.0)
        pb = parm.tile([128, NCC], f32, tag="pb", name="pb")
        nc.vector.reciprocal(pb[:], en[:])

        # ---------------- phase 1: factorized soft match ----------------
        bsb = parm.tile([128, 3 * NCC], f32, tag="bsb", name="bsb")
        with tc.tile_pool(name="psB", bufs=1,
                          space=bass.MemorySpace.PSUM) as psB:
            B_ps = psB.tile([128, 3 * NCC], f32, tag="B", name="B")
            with tc.tile_pool(name="psW", bufs=2,
                              space=bass.MemorySpace.PSUM) as psW:
                for k in range(nch):
                    ct = psW.tile([128, NL], f32, tag="cross", name="cross")
                    nc.tensor.matmul(ct[:], vt_t[:, k * 128:(k + 1) * 128],
                                     rhs_t, start=True, stop=True)
                    wx = work.tile([128, NL], f32, tag="wx", name="wx")
                    nc.scalar.activation(wx[:], ct[:], AF.Exp,
                                         bias=0.0, scale=EXP_SCALE)
                    wze = work.tile([128, 3 * NCC], f32, tag="wze",
                                    name="wze")
                    e3b = e3_t[:, 3 * k:3 * k + 3] \
                        .rearrange("p (one f) -> p one f", one=1) \
                        .broadcast_to([128, NCC, 3])
                    wzb = wx[:, NXY:NL] \
                        .rearrange("p (k one) -> p k one", one=1) \
                        .broadcast_to([128, NCC, 3])
                    nc.vector.tensor_tensor(
                        wze[:].rearrange("p (k f) -> p k f", f=3),
                        e3b, wzb, ALU.mult)
                    nc.tensor.matmul(B_ps[:], wx[:, 0:NXY], wze[:],
                                     start=(k == 0), stop=(k == nch - 1))
            nc.vector.tensor_copy(bsb[:], B_ps[:])
        bs3 = bsb[:].rearrange("p (k f) -> p k f", f=3)

        def pt(tag, w=NCC):
            return parm.tile([128, w], f32, tag=tag, name=tag)

        # ---------------- per-contact params ----------------
        t0 = pt("t0")
        nc.vector.tensor_scalar_add(t0[:], bs3[:, :, 2], 1e-8)
        rws = pt("rws"); nc.vector.reciprocal(rws[:], t0[:])
        pol = pt("pol")
        nc.vector.tensor_mul(pol[:], bs3[:, :, 0], rws[:])
        ecc = pt("ecc")
        nc.vector.tensor_mul(ecc[:], bs3[:, :, 1], rws[:])

        # t20 = [t | |t| - pi/2], t = pol*rad - pi;  poly gives
        # [sin(t) | -cos(t)]  (factored-root form, 6 fused DVE ops).
        t20 = pt("t20", 2 * NCC)
        nc.vector.tensor_scalar(t20[:, 0:NCC], pol[:], _DEG2RAD, -PI,
                                ALU.mult, ALU.add)
        nc.vector.scalar_tensor_tensor(t20[:, NCC:2 * NCC], t20[:, 0:NCC],
                                       -1.0, t20[:, 0:NCC],
                                       ALU.mult, ALU.max)
        nc.vector.tensor_scalar_add(t20[:, NCC:2 * NCC],
                                    t20[:, NCC:2 * NCC], -PI / 2.0)
        px = pt("px", 2 * NCC)
        nc.vector.tensor_mul(px[:], t20[:], t20[:])
        pu = pt("pu", 2 * NCC)
        nc.vector.tensor_scalar_mul(pu[:], t20[:], C4)
        pa = pt("pa", 2 * NCC)
        nc.vector.scalar_tensor_tensor(pa[:], px[:], -R1, pu[:],
                                       ALU.add, ALU.mult)
        pb2 = pt("pb2", 2 * NCC)
        nc.vector.scalar_tensor_tensor(pb2[:], px[:], -R2, pa[:],
                                       ALU.add, ALU.mult)
        pq = pt("pq", 2 * NCC)
        nc.vector.scalar_tensor_tensor(pq[:], px[:], QB, px[:],
                                       ALU.add, ALU.mult)
        sc20 = pt("sc20", 2 * NCC)
        nc.vector.scalar_tensor_tensor(sc20[:], pq[:], QC, pb2[:],
                                       ALU.add, ALU.mult)

        # t12 = ecc * [sn | -cs];  nxy = t12*SE + [-128 | -127]
        t12 = pt("t12", 2 * NCC)
        eb = ecc[:].rearrange("p (one k) -> p one k", one=1) \
            .broadcast_to([128, 2, NCC])
        nc.vector.tensor_tensor(t12[:].rearrange("p (two k) -> p two k",
                                                 two=2),
                                sc20[:].rearrange("p (two k) -> p two k",
                                                  two=2),
                                eb, ALU.mult)
        nxy = pt("nxy", 2 * NCC)
        nc.vector.scalar_tensor_tensor(nxy[:], t12[:], SE, konst[:],
                                       ALU.mult, ALU.add)

        # 3-level hi/lo center split + squared-head coeffs into the
        # [128, 100] coeff matrix (j = [1,1,syh,syl,syl2,sysh,
        # sxh,sxl,sxl2,sxsh]; halves of nxy are [x | y]).
        # DVE does the center splits; GpSimd does the square chain.
        hi16 = parm.tile([128, 2 * NCC], f16, tag="hi16", name="hi16")
        nc.vector.tensor_copy(hi16[:], nxy[:])
        nc.vector.tensor_copy(cj[:, :, 2], hi16[:, NCC:2 * NCC])
        nc.vector.tensor_copy(cj[:, :, 6], hi16[:, 0:NCC])
        lo1 = pt("lo1", 2 * NCC)
        nc.vector.tensor_sub(lo1[:], nxy[:], hi16[:])
        lo16 = parm.tile([128, 2 * NCC], f16, tag="lo16", name="lo16")
        nc.vector.tensor_copy(lo16[:], lo1[:])
        nc.vector.tensor_copy(cj[:, :, 3], lo16[:, NCC:2 * NCC])
        nc.vector.tensor_copy(cj[:, :, 7], lo16[:, 0:NCC])
        nc.vector.tensor_sub(cj[:, :, 4], lo1[:, NCC:2 * NCC],
                             lo16[:, NCC:2 * NCC])
        nc.vector.tensor_sub(cj[:, :, 8], lo1[:, 0:NCC], lo16[:, 0:NCC])
        sqs = pt("sqs", 2 * NCC)
        nc.gpsimd.tensor_mul(sqs[:], nxy[:], nxy[:])
        sqh16 = parm.tile([128, 2 * NCC], f16, tag="sqh16", name="sqh16")
        nc.gpsimd.tensor_copy(sqh16[:], sqs[:])
        nc.gpsimd.tensor_copy(cj[:, :, 5], sqh16[:, NCC:2 * NCC])
        nc.gpsimd.tensor_copy(cj[:, :, 9], sqh16[:, 0:NCC])
        sql = pt("sql", 2 * NCC)
        nc.gpsimd.tensor_sub(sql[:], sqs[:], sqh16[:])
        s2s = pt("s2s")
        nc.gpsimd.tensor_add(s2s[:], sql[:, 0:NCC], sql[:, NCC:2 * NCC])
        wfac = pt("wfac")
        nc.scalar.activation(wfac[:], s2s[:], AF.Exp, bias=0.0,
                             scale=-1.0 / (1.0 + 1e-8))
        val = pt("val")
        nc.vector.tensor_scalar_min(val[:], bs3[:, :, 2], 1.0)
        wc = pt("wc"); nc.vector.tensor_mul(wc[:], pb[:], val[:])
        wc2 = pt("wc2"); nc.vector.tensor_mul(wc2[:], wc[:], wfac[:])

        lhsT = parm.tile([NJ * NCC, 128], f16, tag="lhsT", name="lhsT")
        with tc.tile_pool(name="psT", bufs=1,
                          space=bass.MemorySpace.PSUM) as psT:
            coefT = psT.tile([NJ * NCC, 128], f32, tag="coefT",
                             name="coefT")
            nc.tensor.transpose(coefT[:], ctile[:], eye_t)
            nc.vector.tensor_copy(lhsT[:], coefT[:])

        # HAM insurance: data-dependent PE work (cannot be hoisted before
        # phase 1) keeps the clock gate at 2.4 GHz through the params
        # window so phase 2 runs fast.
        for _ in range(3):
            nc.tensor.matmul(wps[0:30, 0:30], bsb[:], bsb[:],
                             start=True, stop=True, skip_group_check=True)
        for _ in range(4):
            nc.tensor.matmul(wps[0:2 * NCC, 0:2 * NCC], hi16[:], hi16[:],
                             start=True, stop=True, skip_group_check=True)

        # ---------------- phase 2: separable splat (chunk pairs) --------
        psM = ctx.enter_context(
            tc.tile_pool(name="psM", bufs=1, space=bass.MemorySpace.PSUM))
        psY = ctx.enter_context(
            tc.tile_pool(name="psY", bufs=2, space=bass.MemorySpace.PSUM))
        mp = psM.tile([128, 2 * MAP_SIZE], f32, tag="map", name="map")
        SC_E = -1.0 / (1.0 + 1e-8)

        def consume(p, yp):
            xy8 = work.tile([128, 4 * MAP_SIZE], f16, tag="xy8", name="xy8")
            nc.scalar.activation(xy8[:], yp[:], AF.Exp, bias=0.0,
                                 scale=SC_E)
            for h in (0, 1):
                c = 2 * p + h
                o = 2 * MAP_SIZE * h
                yy = work.tile([128, MAP_SIZE], f16, tag="yy", name="yy")
                nc.vector.tensor_scalar_mul(yy[:], xy8[:, o:o + MAP_SIZE],
                                            wc2[:, c:c + 1])
                xx = xy8[:, o + MAP_SIZE:o + 2 * MAP_SIZE]
                nc.tensor.matmul(mp[:, 0:MAP_SIZE], yy[:, 0:128], xx,
                                 start=(c == 0), stop=(c == NCC - 1))
                nc.tensor.matmul(mp[:, MAP_SIZE:2 * MAP_SIZE],
                                 yy[:, 128:256], xx,
                                 start=(c == 0), stop=(c == NCC - 1))

        prev = None
        for p in range(NCC // 2):
            yp = psY.tile([128, 4 * MAP_SIZE], f32, tag="yp", name="yp")
            for h in (0, 1):
                c = 2 * p + h
                nc.tensor.matmul(
                    yp[:, 512 * h:512 * (h + 1)], lhsT[:],
                    bas_t[:, 512 * c:512 * (c + 1)],
                    start=True, stop=True)
            if prev is not None:
                consume(*prev)
            prev = (p, yp)
        consume(*prev)

        # ---------------- normalize + store ----------------
        mx = parm.tile([128, 1], f32, tag="mx", name="mx")
        nc.vector.reduce_max(mx[:], mp[:], axis=mybir.AxisListType.X)
        with tc.tile_pool(name="psG", bufs=1,
                          space=bass.MemorySpace.PSUM) as psG:
            mt = psG.tile([1, 128], f32, tag="mt", name="mt")
            nc.tensor.transpose(mt[:], mx[:], eye_t)
            gm = parm.tile([1, 1], f32, tag="gm", name="gm")
            nc.vector.reduce_max(gm[:], mt[:], axis=mybir.AxisListType.X)
            nc.vector.tensor_scalar_add(gm[:], gm[:], 1e-8)
            gi = parm.tile([1, 1], f32, tag="gi", name="gi")
            nc.vector.reciprocal(gi[:], gm[:])
            gi16 = parm.tile([1, 1], f16, tag="gi16", name="gi16")
            nc.vector.tensor_copy(gi16[:], gi[:])
            gb = psG.tile([128, 1], f32, tag="gb", name="gb")
            nc.tensor.matmul(gb[:], ones16[:], gi16[:],
                             start=True, stop=True)
            gs = parm.tile([128, 1], f32, tag="gs", name="gs")
            nc.vector.tensor_copy(gs[:], gb[:])

        o0 = work.tile([128, MAP_SIZE], f32, tag="o0", name="o0")
        nc.vector.tensor_scalar_mul(o0[:], mp[:, 0:MAP_SIZE], gs[:])
        o1 = work.tile([128, MAP_SIZE], f32, tag="o1", name="o1")
        nc.scalar.activation(o1[:], mp[:, MAP_SIZE:2 * MAP_SIZE],
                             AF.Copy, scale=gs[:])
        nc.sync.dma_start(out_d[0:128, :], o0[:])
        nc.scalar.dma_start(out_d[128:256, :], o1[:])
    return nc


# ----------------------------------------------------------------- entry
def _run(inputs, trace=False):
    params = np.asarray(inputs["params"], np.float32)
    logits = np.asarray(inputs["electrode_logits"], np.float32)
    v1_pos = np.asarray(inputs["v1_pos"], np.float32)
    v1_prf = np.asarray(inputs["v1_prf"], np.float32)
    start_loc = np.asarray(inputs["start_loc"], np.float32)
    surf_dist_lut = np.asarray(inputs["surf_dist_lut"], np.float32)
    alpha_grid = np.asarray(inputs["alpha_grid"], np.float32)
    beta_grid = np.asarray(inputs["beta_grid"], np.float32)

    gc, R, direction, shank = _host_geometry(
        params, start_loc, surf_dist_lut, alpha_grid, beta_grid)
    keeps = [_voxel_keep(v1_pos, gc[b], R[b, :, 2], shank[b] / 2.0)
             for b in range(B)]
    nkeep = max(int(k.sum()) for k in keeps)
    VP = max(128, ((nkeep + 127) // 128) * 128)

    in_maps = []
    for b in range(B):
        k = keeps[b]
        in_maps.append(_prep_core(gc[b], R[b], shank[b], logits[b],
                                  v1_pos[k], v1_prf[k], VP))
    nc = _build_nc(VP)
    _split_multiwaits(nc)
    res = run_bass_kernel_spmd(nc, in_maps, list(range(B)), trace=trace)
    out = np.stack([res.results[i]["out"] for i in range(B)])
    return out[:, None, :, :].astype(np.float32), res


def kernel(**inputs) -> np.ndarray:
    out, _ = _run(inputs, trace=False)
    return out


# revision 33
# speedup vs baseline: 1.1158x; 1.0288x over previous
"""Trainium2 Bass kernel for nn_DifferentiableSimulator.

Strategy (8 NeuronCores, B=8): one batch element per core, no collectives.

Host side (cheap, O(V+N)):
  - per-batch probe geometry: rotation, LUT bilinear interp (tiny)
  - per-batch voxel relevance sharding: keep voxels within CUT(8mm) +
    probe-radius of the shank axis segment (dropped voxels are suppressed
    by >= e^-9.6 relative to any weight that can influence a visible
    pixel; empirically the output matches the dense reference to ~2e-4).
    Keeps ~850 of 10k voxels -> 7 chunks of 128.
  - lattice factorization: the 1000 contacts are a rigid 10x10x10 grid,
    so in the rotated frame the soft-match weight matrix factorizes as
    W[n,v] = Wxy[(ij),v] * Wz[k,v]: 138 gaussian columns per voxel
    instead of 1000.  Voxel features ship as fp16 hi/lo pairs so the
    fp16 matmul is ~fp32-exact.
  - contacts are reindexed m = k*128 + (iy*10+ix) (28 dummy xy slots per
    z-layer, weight 0) so weighted sums land in contact-chunk layout.
  - In this input regime phos_size == 1 exactly (m_inv <= 0.82 =>
    sigma*SE <= 0.45 < 1 for any ecc in [0,12], which the weighted mean
    guarantees), so the splat gaussian args are just (i - center)^2 and
    the whole sigma-reciprocal chain drops out.

Device side (per core), phase 1 -- soft PRF match per 128-voxel chunk:
  one K=17 fp16 matmul -> xy/z gaussian exponents [128v, 138] in PSUM;
  ACT exp; one DVE op forms WzE = Wz x [pol, ecc, 1]; one fp32 matmul
  accumulates B[128ij, 30] = all weighted sums, contact-major.

Params: pol/ecc/wsum -> sin/cos via a factored-root polynomial evaluated
  in 6 fused DVE ops (no ACT Sin => the Exp table is loaded once at
  startup and NEVER reloaded).  Splat centers are split hi/lo into a
  [128, 50] coeff matrix, one PE transpose -> [50,128] fp16 lhsT.

Phase 2 -- separable splat.  The SQUARED args are emitted directly by
  the PE: (i+s)^2 = i^2 + 2i*s + s^2 with i^2 shipped as exact f16
  hi/lo basis rows, s as a 3-level f16 split (coeff rows), and s^2 as
  its f16 head; the s^2 residual folds into the per-contact weight via
  one tiny exp (all bounded so no f16 overflow).  One k=100 fp16 matmul
  per chunk (block-diagonal [100, 5120] host basis, DMA'd early on an
  idle queue) -> [128, 512] squared args in PSUM; chunks are consumed in
  pairs: one ACT Exp [128,1024] straight from PSUM -> f16 factors; DVE
  folds wc*wfac into the row factor; four fp16 matmuls per pair
  accumulate the 256x256 map in one [128,512] PSUM bank.

Max via row-reduce + PE transpose; f16 reciprocal broadcast; scale on
DVE + ACT; two output DMAs.  A PE warmup burst at startup plus
data-dependent insurance matmuls (on bsb/hi16, so the scheduler cannot
hoist them) keep the HAM clock gate at 2.4 GHz through phase 2.
"""
import math
from contextlib import ExitStack

import numpy as np

import concourse.bass as bass
import concourse.mybir as mybir
from concourse import bass_isa
from concourse import tile
from concourse.bass_utils import run_bass_kernel_spmd

# ---- constants (must match the reference) ----
_DEG2RAD = math.pi / 180.0
MAP_SIZE = 256
SOFT_MATCH_SIGMA = 1.5

B = 8
NCC = 10                  # contact chunks = z-layers
NXY = 128                 # xy-lattice slots per layer (100 real + 28 dummy)
CUT = 8.0
XY_RAD = 1.8 * math.sqrt(2.0)
SE = MAP_SIZE / 90.0
EXP_SCALE = 2.0 / (2.0 * SOFT_MATCH_SIGMA ** 2)   # 2/4.5
SCALE_EXP2 = -4.0 / (1.0 + 1e-8)                  # undoes the 0.25 in sq

# sin(y) = y*c4*(y^2-R1)*(y^2-R2)*((y^2+QB)*y^2+QC) on [-pi,pi]; err 2e-5
C4 = 2.17323611e-06
R1 = 9.869712469760742
R2 = 29.379373192742293
QB = -49.63325349575811
QC = 1586.862759630811

f32 = mybir.dt.float32
f16 = mybir.dt.float16
AF = mybir.ActivationFunctionType
ALU = mybir.AluOpType
PI = math.pi


# ---------------------------------------------------------------- host prep
def _f16s(x):
    hi = np.float16(x)
    lo = np.float16(np.float32(x) - np.float32(hi))
    return hi, lo


def _f16_split(x):
    hi = x.astype(np.float16)
    lo = (x.astype(np.float32) - hi.astype(np.float32)).astype(np.float16)
    return hi.astype(np.float32), lo.astype(np.float32)


def _host_geometry(params, start_loc, surf_dist_lut, alpha_grid, beta_grid):
    params = params.astype(np.float64)
    alpha, beta, offset, shank = (params[:, 0], params[:, 1],
                                  params[:, 2], params[:, 3])
    a = alpha * _DEG2RAD
    b = beta * _DEG2RAD
    ca, sa = np.cos(a), np.sin(a)
    cb, sb = np.cos(b), np.sin(b)
    Bn = params.shape[0]
    Rx = np.zeros((Bn, 3, 3)); Ry = np.zeros((Bn, 3, 3))
    Rx[:, 0, 0] = 1; Rx[:, 1, 1] = ca; Rx[:, 1, 2] = -sa
    Rx[:, 2, 1] = sa; Rx[:, 2, 2] = ca
    Ry[:, 0, 0] = cb; Ry[:, 0, 2] = sb; Ry[:, 1, 1] = 1
    Ry[:, 2, 0] = -sb; Ry[:, 2, 2] = cb
    R = Rx @ Ry
    direction = np.einsum('bij,j->bi', R, np.array([0.0, 0.0, -1.0]))
    direction = direction / np.linalg.norm(direction, axis=-1, keepdims=True)
    lut = surf_dist_lut.astype(np.float64)
    na, nb = lut.shape
    ag, bg = alpha_grid.astype(np.float64), beta_grid.astype(np.float64)
    a_norm = 2.0 * (alpha - ag[0]) / (ag[-1] - ag[0] + 1e-08) - 1.0
    b_norm = 2.0 * (beta - bg[0]) / (bg[-1] - bg[0] + 1e-08) - 1.0
    ai = np.clip((a_norm + 1.0) * 0.5 * (na - 1), 0.0, na - 1.0)
    bi = np.clip((b_norm + 1.0) * 0.5 * (nb - 1), 0.0, nb - 1.0)
    a0 = np.clip(np.floor(ai), 0, na - 1).astype(np.int64)
    b0 = np.clip(np.floor(bi), 0, nb - 1).astype(np.int64)
    a1 = np.minimum(a0 + 1, na - 1)
    b1 = np.minimum(b0 + 1, nb - 1)
    fa = ai - a0
    fb = bi - b0
    v00 = lut[a0, b0]; v01 = lut[a0, b1]; v10 = lut[a1, b0]; v11 = lut[a1, b1]
    surf = (v00 * (1 - fa) * (1 - fb) + v01 * (1 - fa) * fb
            + v10 * fa * (1 - fb) + v11 * fa * fb)
    surf = np.maximum(surf, 1.0)
    penetration = surf - shank / 2.0 - offset
    grid_center = (start_loc.astype(np.float64)[None, :]
                   + direction * penetration[:, None])
    return grid_center, R, direction, shank


def _voxel_keep(v1_pos, grid_center, axis_dir, half_len):
    d = v1_pos.astype(np.float64) - grid_center[None, :]
    t = np.clip(d @ axis_dir, -half_len, half_len)
    dist = np.linalg.norm(d - t[:, None] * axis_dir[None, :], axis=1)
    return dist <= (CUT + XY_RAD + 0.5)


NJ = 10     # coeff rows per chunk -> k = 100


def _basis100():
    """[100, 512*NCC] f16 block-diagonal squared-args basis.  Chunk c's
    rhs is the [100, 512] column block; only rows 10c..10c+10 are nonzero
    there, pairing with lhsT coeff rows
    [1, 1, sy_h, sy_l, sy_l2, sys_h, sx_h, sx_l, sx_l2, sxs_h]:
      arg_y(i) = i2_h + i2_l + 2i*(sy_h+sy_l+sy_l2) + sys_h   (cols 0:256)
      arg_x(i) = likewise with sx/sxs                         (cols 256:512)
    i^2 = i2_h + i2_l exactly in f16; 2i <= 510 is f16-exact."""
    i = np.arange(MAP_SIZE, dtype=np.float64)
    i2 = i * i
    i2h = i2.astype(np.float16).astype(np.float64)
    i2l = i2 - i2h
    z = np.zeros(MAP_SIZE)
    o = np.ones(MAP_SIZE)
    ti = 2.0 * i
    blk = np.stack([np.concatenate([i2h, i2h]),
                    np.concatenate([i2l, i2l]),
                    np.concatenate([ti, z]), np.concatenate([ti, z]),
                    np.concatenate([ti, z]), np.concatenate([o, z]),
                    np.concatenate([z, ti]), np.concatenate([z, ti]),
                    np.concatenate([z, ti]), np.concatenate([z, o])])
    bas = np.zeros((NJ * NCC, 512 * NCC), np.float64)
    for c in range(NCC):
        bas[NJ * c:NJ * (c + 1), 512 * c:512 * (c + 1)] = blk
    return bas.astype(np.float16)


def _prep_core(gc_b, R_b, shank_b, logits_b, v1_pos_k, v1_prf_k, VP):
    """Per-core device input arrays (packed into two DMA blocks)."""
    Vk = v1_pos_k.shape[0]
    nch = VP // 128
    w = np.zeros((VP, 3))
    w[:Vk] = (v1_pos_k.astype(np.float64) - gc_b[None, :]) @ R_b
    wf = w.astype(np.float32)
    wh, wl = _f16_split(wf)
    bxy = (-0.5 * (w[:, 0] ** 2 + w[:, 1] ** 2)).astype(np.float32)
    bz = (-0.5 * w[:, 2] ** 2).astype(np.float32)
    bxy[Vk:] = -30000.0
    bz[Vk:] = -30000.0
    bxyh, bxyl = _f16_split(bxy)
    bzh, bzl = _f16_split(bz)
    onesv = np.ones(VP, np.float32)
    vt = np.stack([wh[:, 0], wh[:, 1], wl[:, 0], wl[:, 1], wh[:, 0],
                   wh[:, 1], onesv, onesv, bxyh, bxyl,
                   wh[:, 2], wl[:, 2], wh[:, 2], onesv, onesv, bzh, bzl],
                  axis=0).astype(np.float16)

    xs = np.arange(10) * 0.4 - 1.8
    zs = (np.linspace(0.0, 1.0, 10) - 0.5) * float(shank_b)
    cols = np.zeros((17, NXY + 10), np.float32)
    for ij in range(NXY):
        if ij < 100:
            iy, ix = ij // 10, ij % 10
            x, y = xs[ix], xs[iy]
            xh, xl = _f16s(x)
            yh, yl = _f16s(y)
            axyh, axyl = _f16s(-0.5 * (x * x + y * y))
            cols[0:10, ij] = [xh, yh, xh, yh, xl, yl, axyh, axyl, 1.0, 1.0]
        else:
            cols[6, ij] = -30000.0     # dummy xy slot -> Wxy = 0
            cols[8, ij] = 1.0
    for k in range(10):
        z = zs[k]
        zh, zl = _f16s(z)
        azh, azl = _f16s(-0.5 * z * z)
        cols[10:17, NXY + k] = [zh, zh, zl, azh, azl, 1.0, 1.0]
    rhs = cols.astype(np.float16)

    d16 = np.concatenate([vt, rhs], axis=1)        # [17, VP+138]

    e3 = np.zeros((VP, 3), np.float32)
    e3[:Vk, 0] = v1_prf_k[:, 0]
    e3[:Vk, 1] = v1_prf_k[:, 1]
    e3[:Vk, 2] = 1.0
    e3t = np.ascontiguousarray(
        e3.reshape(nch, 128, 3).transpose(1, 0, 2).reshape(128, 3 * nch))

    lgt = np.full((NXY, NCC), -30.0, np.float32)
    iy, ix = np.divmod(np.arange(100), 10)
    for k in range(NCC):
        lgt[:100, k] = logits_b[iy * 100 + ix * 10 + k]
    d32 = np.concatenate([e3t, lgt, np.eye(128, dtype=np.float32)], axis=1)
    return {"d16": np.ascontiguousarray(d16),
            "d32": np.ascontiguousarray(d32),
            "bas": _basis100()}


# ------------------------------------------------------------- device kernel
def _split_multiwaits(nc):
    """This walrus build accepts at most ONE sync wait per instruction.
    Tile emits several.  Engine instruction streams execute in order, so
    moving all but one wait onto single-wait NoOps inserted just before
    the instruction preserves semantics exactly."""
    cnt = 0
    for fn in nc.m.functions:
        for blk in fn.blocks:
            out = []
            for inst in blk.instructions:
                si = inst.sync_info
                if si is not None and si.on_wait is not None \
                        and len(si.on_wait) > 1:
                    waits = list(si.on_wait)
                    for w in waits[:-1]:
                        cnt += 1
                        out.append(mybir.InstNoOp(
                            name=f"WSPLIT-{cnt}",
                            engine=inst.engine,
                            ins=[], outs=[],
                            sync_info=mybir.SyncInfo(on_wait=[w],
                                                     on_update=[]),
                        ))
                    inst.sync_info = mybir.SyncInfo(
                        on_wait=[waits[-1]], on_update=list(si.on_update))
                out.append(inst)
            blk.instructions = out
    return cnt


def _build_nc(VP):
    nch = VP // 128
    NL = NXY + 10          # 138 lattice columns
    W16 = VP + NL
    W32 = 3 * nch + NCC + 128
    nc = bass.Bass()
    d16_d = nc.dram_tensor("d16", [17, W16], f16, kind="ExternalInput")
    d32_d = nc.dram_tensor("d32", [128, W32], f32, kind="ExternalInput")
    bas_d = nc.dram_tensor("bas", [NJ * NCC, 512 * NCC], f16,
                           kind="ExternalInput")
    out_d = nc.dram_tensor("out", [MAP_SIZE, MAP_SIZE], f32,
                           kind="ExternalOutput")

    with ExitStack() as ctx:
        tc = ctx.enter_context(tile.TileContext(nc))
        constp = ctx.enter_context(tc.tile_pool(name="const", bufs=1))
        parm = ctx.enter_context(tc.tile_pool(name="parm", bufs=1))
        work = ctx.enter_context(tc.tile_pool(name="work", bufs=6))
        psA = ctx.enter_context(
            tc.tile_pool(name="psA", bufs=1, space=bass.MemorySpace.PSUM))

        # Warmups first (top scheduler priority): ACT exp-table load + PE
        # HAM burst run during the sem-init + input-DMA window.
        scr = constp.tile([1, 1], f32, tag="scr", name="scr")
        nc.vector.memset(scr[:], 0.0)
        nc.scalar.activation(scr[:], scr[:], AF.Exp, bias=0.0, scale=1.0)
        wrm = constp.tile([128, 512], f16, tag="wrm", name="wrm")
        nc.vector.memset(wrm[:], 0.0)
        wps = psA.tile([128, 512], f32, tag="wps", name="wps")
        for _ in range(12):
            nc.tensor.matmul(wps[:], wrm[:, 0:128], wrm[:],
                             start=True, stop=True, skip_group_check=True)

        bas_t = constp.tile([NJ * NCC, 512 * NCC], f16, tag="bas",
                            name="bas")
        nc.gpsimd.dma_start(bas_t[:], bas_d[:])
        dp16 = constp.tile([17, W16], f16, tag="dp16", name="dp16")
        nc.sync.dma_start(dp16[:], d16_d[:])
        dp32 = constp.tile([128, W32], f32, tag="dp32", name="dp32")
        nc.scalar.dma_start(dp32[:], d32_d[:])
        vt_t = dp16[:, 0:VP]
        rhs_t = dp16[:, VP:VP + NL]
        e3_t = dp32[:, 0:3 * nch]
        lg_t = dp32[:, 3 * nch:3 * nch + NCC]
        eye_t = dp32[:, 3 * nch + NCC:W32]

        ones16 = constp.tile([1, 128], f16, tag="ones16", name="ones16")
        nc.vector.memset(ones16[:], 1.0)
        konst = constp.tile([128, 2 * NCC], f32, tag="konst", name="konst")
        nc.vector.memset(konst[:, 0:NCC], -128.0)
        nc.vector.memset(konst[:, NCC:2 * NCC], -127.0)
        ctile = constp.tile([128, NJ * NCC], f32, tag="ctile", name="ctile")
        cj = ctile[:].rearrange("p (c j) -> p c j", j=NJ)
        nc.vector.memset(cj[:, :, 0], 1.0)
        nc.vector.memset(cj[:, :, 1], 1.0)

        # sigmoid(logits): independent of phase 1, runs early.
        en = parm.tile([128, NCC], f32, tag="en", name="en")
        nc.scalar.activation(en[:], lg_t, AF.Exp, bias=0.0, scale=-1.0)
        nc.vector.tensor_scalar_add(en[:], en[:], 1.0)
        pb = parm.tile([128, NCC], f32, tag="pb", name="pb")
        nc.vector.reciprocal(pb[:], en[:])

        # ---------------- phase 1: factorized soft match ----------------
        bsb = parm.tile([128, 3 * NCC], f32, tag="bsb", name="bsb")
        with tc.tile_pool(name="psB", bufs=1,
                          space=bass.MemorySpace.PSUM) as psB:
            B_ps = psB.tile([128, 3 * NCC], f32, tag="B", name="B")
            with tc.tile_pool(name="psW", bufs=2,
                              space=bass.MemorySpace.PSUM) as psW:
                for k in range(nch):
                    ct = psW.tile([128, NL], f32, tag="cross", name="cross")
                    nc.tensor.matmul(ct[:], vt_t[:, k * 128:(k + 1) * 128],
                                     rhs_t, start=True, stop=True)
                    wx = work.tile([128, NL], f32, tag="wx", name="wx")
                    nc.scalar.activation(wx[:], ct[:], AF.Exp,
                                         bias=0.0, scale=EXP_SCALE)
                    wze = work.tile([128, 3 * NCC], f32, tag="wze",
                                    name="wze")
                    e3b = e3_t[:, 3 * k:3 * k + 3] \
                        .rearrange("p (one f) -> p one f", one=1) \
                        .broadcast_to([128, NCC, 3])
                    wzb = wx[:, NXY:NL] \
                        .rearrange("p (k one) -> p k one", one=1) \
                        .broadcast_to([128, NCC, 3])
                    nc.vector.tensor_tensor(
                        wze[:].rearrange("p (k f) -> p k f", f=3),
                        e3b, wzb, ALU.mult)
                    nc.tensor.matmul(B_ps[:], wx[:, 0:NXY], wze[:],
                                     start=(k == 0), stop=(k == nch - 1))
            nc.vector.tensor_copy(bsb[:], B_ps[:])
        bs3 = bsb[:].rearrange("p (k f) -> p k f", f=3)

        def pt(tag, w=NCC):
            return parm.tile([128, w], f32, tag=tag, name=tag)

        # ---------------- per-contact params ----------------
        t0 = pt("t0")
        nc.vector.tensor_scalar_add(t0[:], bs3[:, :, 2], 1e-8)
        rws = pt("rws"); nc.vector.reciprocal(rws[:], t0[:])
        pol = pt("pol")
        nc.vector.tensor_mul(pol[:], bs3[:, :, 0], rws[:])
        ecc = pt("ecc")
        nc.vector.tensor_mul(ecc[:], bs3[:, :, 1], rws[:])

        # t20 = [t | |t| - pi/2], t = pol*rad - pi;  poly gives
        # [sin(t) | -cos(t)]  (factored-root form, 6 fused DVE ops).
        t20 = pt("t20", 2 * NCC)
        nc.vector.tensor_scalar(t20[:, 0:NCC], pol[:], _DEG2RAD, -PI,
                                ALU.mult, ALU.add)
        nc.vector.scalar_tensor_tensor(t20[:, NCC:2 * NCC], t20[:, 0:NCC],
                                       -1.0, t20[:, 0:NCC],
                                       ALU.mult, ALU.max)
        nc.vector.tensor_scalar_add(t20[:, NCC:2 * NCC],
                                    t20[:, NCC:2 * NCC], -PI / 2.0)
        px = pt("px", 2 * NCC)
        nc.vector.tensor_mul(px[:], t20[:], t20[:])
        pu = pt("pu", 2 * NCC)
        nc.vector.tensor_scalar_mul(pu[:], t20[:], C4)
        pa = pt("pa", 2 * NCC)
        nc.vector.scalar_tensor_tensor(pa[:], px[:], -R1, pu[:],
                                       ALU.add, ALU.mult)
        pb2 = pt("pb2", 2 * NCC)
        nc.vector.scalar_tensor_tensor(pb2[:], px[:], -R2, pa[:],
                                       ALU.add, ALU.mult)
        pq = pt("pq", 2 * NCC)
        nc.vector.scalar_tensor_tensor(pq[:], px[:], QB, px[:],
                                       ALU.add, ALU.mult)
        sc20 = pt("sc20", 2 * NCC)
        nc.vector.scalar_tensor_tensor(sc20[:], pq[:], QC, pb2[:],
                                       ALU.add, ALU.mult)

        # t12 = ecc * [sn | -cs];  nxy = t12*SE + [-128 | -127]
        t12 = pt("t12", 2 * NCC)
        eb = ecc[:].rearrange("p (one k) -> p one k", one=1) \
            .broadcast_to([128, 2, NCC])
        nc.vector.tensor_tensor(t12[:].rearrange("p (two k) -> p two k",
                                                 two=2),
                                sc20[:].rearrange("p (two k) -> p two k",
                                                  two=2),
                                eb, ALU.mult)
        nxy = pt("nxy", 2 * NCC)
        nc.vector.scalar_tensor_tensor(nxy[:], t12[:], SE, konst[:],
                                       ALU.mult, ALU.add)

        # 3-level hi/lo center split + squared-head coeffs into the
        # [128, 100] coeff matrix (j = [1,1,syh,syl,syl2,sysh,
        # sxh,sxl,sxl2,sxsh]; halves of nxy are [x | y]).
        # DVE does the center splits; GpSimd does the square chain.
        hi16 = parm.tile([128, 2 * NCC], f16, tag="hi16", name="hi16")
        nc.vector.tensor_copy(hi16[:], nxy[:])
        nc.vector.tensor_copy(cj[:, :, 2], hi16[:, NCC:2 * NCC])
        nc.vector.tensor_copy(cj[:, :, 6], hi16[:, 0:NCC])
        lo1 = pt("lo1", 2 * NCC)
        nc.vector.tensor_sub(lo1[:], nxy[:], hi16[:])
        lo16 = parm.tile([128, 2 * NCC], f16, tag="lo16", name="lo16")
        nc.vector.tensor_copy(lo16[:], lo1[:])
        nc.vector.tensor_copy(cj[:, :, 3], lo16[:, NCC:2 * NCC])
        nc.vector.tensor_copy(cj[:, :, 7], lo16[:, 0:NCC])
        nc.vector.tensor_sub(cj[:, :, 4], lo1[:, NCC:2 * NCC],
                             lo16[:, NCC:2 * NCC])
        nc.vector.tensor_sub(cj[:, :, 8], lo1[:, 0:NCC], lo16[:, 0:NCC])
        sqs = pt("sqs", 2 * NCC)
        nc.gpsimd.tensor_mul(sqs[:], nxy[:], nxy[:])
        sqh16 = parm.tile([128, 2 * NCC], f16, tag="sqh16", name="sqh16")
        nc.gpsimd.tensor_copy(sqh16[:], sqs[:])
        nc.gpsimd.tensor_copy(cj[:, :, 5], sqh16[:, NCC:2 * NCC])
        nc.gpsimd.tensor_copy(cj[:, :, 9], sqh16[:, 0:NCC])
        sql = pt("sql", 2 * NCC)
        nc.gpsimd.tensor_sub(sql[:], sqs[:], sqh16[:])
        s2s = pt("s2s")
        nc.gpsimd.tensor_add(s2s[:], sql[:, 0:NCC], sql[:, NCC:2 * NCC])
        wfac = pt("wfac")
        nc.scalar.activation(wfac[:], s2s[:], AF.Exp, bias=0.0,
                             scale=-1.0 / (1.0 + 1e-8))
        val = pt("val")
        nc.vector.tensor_scalar_min(val[:], bs3[:, :, 2], 1.0)
        wc = pt("wc"); nc.vector.tensor_mul(wc[:], pb[:], val[:])
        wc2 = pt("wc2"); nc.vector.tensor_mul(wc2[:], wc[:], wfac[:])

        lhsT = parm.tile([NJ * NCC, 128], f16, tag="lhsT", name="lhsT")
        with tc.tile_pool(name="psT", bufs=1,
                          space=bass.MemorySpace.PSUM) as psT:
            coefT = psT.tile([NJ * NCC, 128], f32, tag="coefT",
                             name="coefT")
            nc.tensor.transpose(coefT[:], ctile[:], eye_t)
            nc.vector.tensor_copy(lhsT[:], coefT[:])

        # HAM insurance: data-dependent PE work (cannot be hoisted before
        # phase 1) keeps the clock gate at 2.4 GHz through the params
        # window so phase 2 runs fast.  Sized to fill the ~4us window.
        for _ in range(6):
            nc.tensor.matmul(wps[0:30, 0:30], bsb[:], bsb[:],
                             start=True, stop=True, skip_group_check=True)
        for _ in range(6):
            nc.tensor.matmul(wps[0:2 * NCC, 0:2 * NCC], hi16[:], hi16[:],
                             start=True, stop=True, skip_group_check=True)

        # ---------------- phase 2: separable splat (chunk pairs) --------
        psM = ctx.enter_context(
            tc.tile_pool(name="psM", bufs=1, space=bass.MemorySpace.PSUM))
        psY = ctx.enter_context(
            tc.tile_pool(name="psY", bufs=2, space=bass.MemorySpace.PSUM))
        mp = psM.tile([128, 2 * MAP_SIZE], f32, tag="map", name="map")
        SC_E = -1.0 / (1.0 + 1e-8)

        def consume(p, yp):
            xy8 = work.tile([128, 4 * MAP_SIZE], f16, tag="xy8", name="xy8")
            nc.scalar.activation(xy8[:], yp[:], AF.Exp, bias=0.0,
                                 scale=SC_E)
            for h in (0, 1):
                c = 2 * p + h
                o = 2 * MAP_SIZE * h
                yy = work.tile([128, MAP_SIZE], f16, tag="yy", name="yy")
                nc.vector.tensor_scalar_mul(yy[:], xy8[:, o:o + MAP_SIZE],
                                            wc2[:, c:c + 1])
                xx = xy8[:, o + MAP_SIZE:o + 2 * MAP_SIZE]
                nc.tensor.matmul(mp[:, 0:MAP_SIZE], yy[:, 0:128], xx,
                                 start=(c == 0), stop=(c == NCC - 1))
                nc.tensor.matmul(mp[:, MAP_SIZE:2 * MAP_SIZE],
                                 yy[:, 128:256], xx,
                                 start=(c == 0), stop=(c == NCC - 1))

        prev = None
        for p in range(NCC // 2):
            yp = psY.tile([128, 4 * MAP_SIZE], f32, tag="yp", name="yp")
            for h in (0, 1):
                c = 2 * p + h
                nc.tensor.matmul(
                    yp[:, 512 * h:512 * (h + 1)], lhsT[:],
                    bas_t[:, 512 * c:512 * (c + 1)],
                    start=True, stop=True)
            if prev is not None:
                consume(*prev)
            prev = (p, yp)
        consume(*prev)

        # ---------------- normalize + store ----------------
        mx = parm.tile([128, 1], f32, tag="mx", name="mx")
        nc.vector.reduce_max(mx[:], mp[:], axis=mybir.AxisListType.X)
        with tc.tile_pool(name="psG", bufs=1,
                          space=bass.MemorySpace.PSUM) as psG:
            mt = psG.tile([1, 128], f32, tag="mt", name="mt")
            nc.tensor.transpose(mt[:], mx[:], eye_t)
            gm = parm.tile([1, 1], f32, tag="gm", name="gm")
            nc.vector.reduce_max(gm[:], mt[:], axis=mybir.AxisListType.X)
            nc.vector.tensor_scalar_add(gm[:], gm[:], 1e-8)
            gi16 = parm.tile([1, 1], f16, tag="gi16", name="gi16")
            with nc.allow_low_precision(reason="norm factor, 5e-4 is fine"):
                nc.vector.reciprocal(gi16[:], gm[:])
            gb = psG.tile([128, 1], f32, tag="gb", name="gb")
            nc.tensor.matmul(gb[:], ones16[:], gi16[:],
                             start=True, stop=True)
            gs = parm.tile([128, 1], f32, tag="gs", name="gs")
            nc.vector.tensor_copy(gs[:], gb[:])

        o0 = work.tile([128, MAP_SIZE], f32, tag="o0", name="o0")
        nc.vector.tensor_scalar_mul(o0[:], mp[:, 0:MAP_SIZE], gs[:])
        o1 = work.tile([128, MAP_SIZE], f32, tag="o1", name="o1")
        nc.scalar.activation(o1[:], mp[:, MAP_SIZE:2 * MAP_SIZE],
                             AF.Copy, scale=gs[:])
        nc.sync.dma_start(out_d[0:128, :], o0[:])
        nc.scalar.dma_start(out_d[128:256, :], o1[:])
    return nc


# ----------------------------------------------------------------- entry
def _run(inputs, trace=False):
    params = np.asarray(inputs["params"], np.float32)
    logits = np.asarray(inputs["electrode_logits"], np.float32)
    v1_pos = np.asarray(inputs["v1_pos"], np.float32)
    v1_prf = np.asarray(inputs["v1_prf"], np.float32)
    start_loc = np.asarray(inputs["start_loc"], np.float32)
    surf_dist_lut = np.asarray(inputs["surf_dist_lut"], np.float32)
    alpha_grid = np.asarray(inputs["alpha_grid"], np.float32)
    beta_grid = np.asarray(inputs["beta_grid"], np.float32)

    gc, R, direction, shank = _host_geometry(
        params, start_loc, surf_dist_lut, alpha_grid, beta_grid)
    keeps = [_voxel_keep(v1_pos, gc[b], R[b, :, 2], shank[b] / 2.0)
             for b in range(B)]
    nkeep = max(int(k.sum()) for k in keeps)
    VP = max(128, ((nkeep + 127) // 128) * 128)

    in_maps = []
    for b in range(B):
        k = keeps[b]
        in_maps.append(_prep_core(gc[b], R[b], shank[b], logits[b],
                                  v1_pos[k], v1_prf[k], VP))
    nc = _build_nc(VP)
    _split_multiwaits(nc)
    res = run_bass_kernel_spmd(nc, in_maps, list(range(B)), trace=trace)
    out = np.stack([res.results[i]["out"] for i in range(B)])
    return out[:, None, :, :].astype(np.float32), res


def kernel(**inputs) -> np.ndarray:
    out, _ = _run(inputs, trace=False)
    return out


# revision 35
# speedup vs baseline: 1.1841x; 1.0612x over previous
"""Trainium2 Bass kernel for nn_DifferentiableSimulator.

Strategy (8 NeuronCores, B=8): one batch element per core, no collectives.

Host side (cheap, O(V+N)):
  - per-batch probe geometry: rotation, LUT bilinear interp (tiny)
  - per-batch voxel relevance sharding: keep voxels within CUT(8mm) +
    probe-radius of the shank axis segment (dropped voxels are suppressed
    by >= e^-9.6 relative to any weight that can influence a visible
    pixel; empirically the output matches the dense reference to ~2e-4).
    Keeps ~850 of 10k voxels -> 7 chunks of 128.
  - lattice factorization: the 1000 contacts are a rigid 10x10x10 grid,
    so in the rotated frame the soft-match weight matrix factorizes as
    W[n,v] = Wxy[(ij),v] * Wz[k,v]: 138 gaussian columns per voxel
    instead of 1000.  Voxel features ship as fp16 hi/lo pairs so the
    fp16 matmul is ~fp32-exact.
  - contacts are reindexed m = k*128 + (iy*10+ix) (28 dummy xy slots per
    z-layer, weight 0) so weighted sums land in contact-chunk layout.
  - In this input regime phos_size == 1 exactly (m_inv <= 0.82 =>
    sigma*SE <= 0.45 < 1 for any ecc in [0,12], which the weighted mean
    guarantees), so the splat gaussian args are just (i - center)^2 and
    the whole sigma-reciprocal chain drops out.

Device side (per core), phase 1 -- soft PRF match per 128-voxel chunk:
  one K=17 fp16 matmul -> xy/z gaussian exponents [128v, 138] in PSUM;
  ACT exp; one DVE op forms WzE = Wz x [pol, ecc, 1]; one fp32 matmul
  accumulates B[128ij, 30] = all weighted sums, contact-major.

Params: pol/ecc/wsum -> sin/cos via a factored-root polynomial evaluated
  in 6 fused DVE ops (no ACT Sin => the Exp table is loaded once at
  startup and NEVER reloaded).  Splat centers are split hi/lo into a
  [128, 50] coeff matrix, one PE transpose -> [50,128] fp16 lhsT.

Phase 2 -- separable splat.  The SQUARED args are emitted directly by
  the PE: (i+s)^2 = i^2 + 2i*s + s^2 with i^2 shipped as exact f16
  hi/lo basis rows, s as a 3-level f16 split (coeff rows), and s^2 as
  its f16 head; the s^2 residual folds into the per-contact weight via
  one tiny exp (all bounded so no f16 overflow).  One k=100 fp16 matmul
  per chunk (block-diagonal [100, 5120] host basis, DMA'd early on an
  idle queue) -> [128, 512] squared args in PSUM; chunks are consumed in
  pairs: one ACT Exp [128,1024] straight from PSUM -> f16 factors; DVE
  folds wc*wfac into the row factor; four fp16 matmuls per pair
  accumulate the 256x256 map in one [128,512] PSUM bank.

Max via row-reduce + PE transpose; f16 reciprocal broadcast; scale on
DVE + ACT; two output DMAs.  A PE warmup burst at startup plus
data-dependent insurance matmuls (on bsb/hi16, so the scheduler cannot
hoist them) keep the HAM clock gate at 2.4 GHz through phase 2.
"""
import math
from contextlib import ExitStack

import numpy as np

import concourse.bass as bass
import concourse.mybir as mybir
from concourse import bass_isa
from concourse import tile
from concourse.bass_utils import run_bass_kernel_spmd

# ---- constants (must match the reference) ----
_DEG2RAD = math.pi / 180.0
MAP_SIZE = 256
SOFT_MATCH_SIGMA = 1.5

B = 8
NCC = 10                  # contact chunks = z-layers
NXY = 128                 # xy-lattice slots per layer (100 real + 28 dummy)
CUT = 8.0
XY_RAD = 1.8 * math.sqrt(2.0)
SE = MAP_SIZE / 90.0
EXP_SCALE = 2.0 / (2.0 * SOFT_MATCH_SIGMA ** 2)   # 2/4.5
SCALE_EXP2 = -4.0 / (1.0 + 1e-8)                  # undoes the 0.25 in sq

# sin(y) = y*c4*(y^2-R1)*(y^2-R2)*((y^2+QB)*y^2+QC) on [-pi,pi]; err 2e-5
C4 = 2.17323611e-06
R1 = 9.869712469760742
R2 = 29.379373192742293
QB = -49.63325349575811
QC = 1586.862759630811

f32 = mybir.dt.float32
f16 = mybir.dt.float16
AF = mybir.ActivationFunctionType
ALU = mybir.AluOpType
PI = math.pi


# ---------------------------------------------------------------- host prep
def _f16s(x):
    hi = np.float16(x)
    lo = np.float16(np.float32(x) - np.float32(hi))
    return hi, lo


def _f16_split(x):
    hi = x.astype(np.float16)
    lo = (x.astype(np.float32) - hi.astype(np.float32)).astype(np.float16)
    return hi.astype(np.float32), lo.astype(np.float32)


def _host_geometry(params, start_loc, surf_dist_lut, alpha_grid, beta_grid):
    params = params.astype(np.float64)
    alpha, beta, offset, shank = (params[:, 0], params[:, 1],
                                  params[:, 2], params[:, 3])
    a = alpha * _DEG2RAD
    b = beta * _DEG2RAD
    ca, sa = np.cos(a), np.sin(a)
    cb, sb = np.cos(b), np.sin(b)
    Bn = params.shape[0]
    Rx = np.zeros((Bn, 3, 3)); Ry = np.zeros((Bn, 3, 3))
    Rx[:, 0, 0] = 1; Rx[:, 1, 1] = ca; Rx[:, 1, 2] = -sa
    Rx[:, 2, 1] = sa; Rx[:, 2, 2] = ca
    Ry[:, 0, 0] = cb; Ry[:, 0, 2] = sb; Ry[:, 1, 1] = 1
    Ry[:, 2, 0] = -sb; Ry[:, 2, 2] = cb
    R = Rx @ Ry
    direction = np.einsum('bij,j->bi', R, np.array([0.0, 0.0, -1.0]))
    direction = direction / np.linalg.norm(direction, axis=-1, keepdims=True)
    lut = surf_dist_lut.astype(np.float64)
    na, nb = lut.shape
    ag, bg = alpha_grid.astype(np.float64), beta_grid.astype(np.float64)
    a_norm = 2.0 * (alpha - ag[0]) / (ag[-1] - ag[0] + 1e-08) - 1.0
    b_norm = 2.0 * (beta - bg[0]) / (bg[-1] - bg[0] + 1e-08) - 1.0
    ai = np.clip((a_norm + 1.0) * 0.5 * (na - 1), 0.0, na - 1.0)
    bi = np.clip((b_norm + 1.0) * 0.5 * (nb - 1), 0.0, nb - 1.0)
    a0 = np.clip(np.floor(ai), 0, na - 1).astype(np.int64)
    b0 = np.clip(np.floor(bi), 0, nb - 1).astype(np.int64)
    a1 = np.minimum(a0 + 1, na - 1)
    b1 = np.minimum(b0 + 1, nb - 1)
    fa = ai - a0
    fb = bi - b0
    v00 = lut[a0, b0]; v01 = lut[a0, b1]; v10 = lut[a1, b0]; v11 = lut[a1, b1]
    surf = (v00 * (1 - fa) * (1 - fb) + v01 * (1 - fa) * fb
            + v10 * fa * (1 - fb) + v11 * fa * fb)
    surf = np.maximum(surf, 1.0)
    penetration = surf - shank / 2.0 - offset
    grid_center = (start_loc.astype(np.float64)[None, :]
                   + direction * penetration[:, None])
    return grid_center, R, direction, shank


def _voxel_keep(v1_pos, grid_center, axis_dir, half_len):
    d = v1_pos.astype(np.float64) - grid_center[None, :]
    t = np.clip(d @ axis_dir, -half_len, half_len)
    dist = np.linalg.norm(d - t[:, None] * axis_dir[None, :], axis=1)
    return dist <= (CUT + XY_RAD + 0.5)


NJ = 10     # coeff rows per chunk -> k = 100


def _basis100():
    """[100, 512*NCC] f16 block-diagonal squared-args basis.  Chunk c's
    rhs is the [100, 512] column block; only rows 10c..10c+10 are nonzero
    there, pairing with lhsT coeff rows
    [1, 1, sy_h, sy_l, sy_l2, sys_h, sx_h, sx_l, sx_l2, sxs_h]:
      arg_y(i) = i2_h + i2_l + 2i*(sy_h+sy_l+sy_l2) + sys_h   (cols 0:256)
      arg_x(i) = likewise with sx/sxs                         (cols 256:512)
    i^2 = i2_h + i2_l exactly in f16; 2i <= 510 is f16-exact."""
    i = np.arange(MAP_SIZE, dtype=np.float64)
    i2 = i * i
    i2h = i2.astype(np.float16).astype(np.float64)
    i2l = i2 - i2h
    z = np.zeros(MAP_SIZE)
    o = np.ones(MAP_SIZE)
    ti = 2.0 * i
    blk = np.stack([np.concatenate([i2h, i2h]),
                    np.concatenate([i2l, i2l]),
                    np.concatenate([ti, z]), np.concatenate([ti, z]),
                    np.concatenate([ti, z]), np.concatenate([o, z]),
                    np.concatenate([z, ti]), np.concatenate([z, ti]),
                    np.concatenate([z, ti]), np.concatenate([z, o])])
    bas = np.zeros((NJ * NCC, 512 * NCC), np.float64)
    for c in range(NCC):
        bas[NJ * c:NJ * (c + 1), 512 * c:512 * (c + 1)] = blk
    return bas.astype(np.float16)


def _prep_core(gc_b, R_b, shank_b, logits_b, v1_pos_k, v1_prf_k, VP):
    """Per-core device input arrays (packed into two DMA blocks)."""
    Vk = v1_pos_k.shape[0]
    nch = VP // 128
    w = np.zeros((VP, 3))
    w[:Vk] = (v1_pos_k.astype(np.float64) - gc_b[None, :]) @ R_b
    wf = w.astype(np.float32)
    wh, wl = _f16_split(wf)
    bxy = (-0.5 * (w[:, 0] ** 2 + w[:, 1] ** 2)).astype(np.float32)
    bz = (-0.5 * w[:, 2] ** 2).astype(np.float32)
    bxy[Vk:] = -30000.0
    bz[Vk:] = -30000.0
    bxyh, bxyl = _f16_split(bxy)
    bzh, bzl = _f16_split(bz)
    onesv = np.ones(VP, np.float32)
    vt = np.stack([wh[:, 0], wh[:, 1], wl[:, 0], wl[:, 1], wh[:, 0],
                   wh[:, 1], onesv, onesv, bxyh, bxyl,
                   wh[:, 2], wl[:, 2], wh[:, 2], onesv, onesv, bzh, bzl],
                  axis=0).astype(np.float16)

    xs = np.arange(10) * 0.4 - 1.8
    zs = (np.linspace(0.0, 1.0, 10) - 0.5) * float(shank_b)
    cols = np.zeros((17, NXY + 10), np.float32)
    for ij in range(NXY):
        if ij < 100:
            iy, ix = ij // 10, ij % 10
            x, y = xs[ix], xs[iy]
            xh, xl = _f16s(x)
            yh, yl = _f16s(y)
            axyh, axyl = _f16s(-0.5 * (x * x + y * y))
            cols[0:10, ij] = [xh, yh, xh, yh, xl, yl, axyh, axyl, 1.0, 1.0]
        else:
            cols[6, ij] = -30000.0     # dummy xy slot -> Wxy = 0
            cols[8, ij] = 1.0
    for k in range(10):
        z = zs[k]
        zh, zl = _f16s(z)
        azh, azl = _f16s(-0.5 * z * z)
        cols[10:17, NXY + k] = [zh, zh, zl, azh, azl, 1.0, 1.0]
    rhs = cols.astype(np.float16)

    d16 = np.concatenate([vt, rhs], axis=1)        # [17, VP+138]

    e3 = np.zeros((VP, 3), np.float32)
    e3[:Vk, 0] = v1_prf_k[:, 0]
    e3[:Vk, 1] = v1_prf_k[:, 1]
    e3[:Vk, 2] = 1.0
    e3t = np.ascontiguousarray(
        e3.reshape(nch, 128, 3).transpose(1, 0, 2).reshape(128, 3 * nch))

    lgt = np.full((NXY, NCC), -30.0, np.float32)
    iy, ix = np.divmod(np.arange(100), 10)
    for k in range(NCC):
        lgt[:100, k] = logits_b[iy * 100 + ix * 10 + k]
    d32 = np.concatenate([e3t, lgt, np.eye(128, dtype=np.float32)], axis=1)
    return {"d16": np.ascontiguousarray(d16),
            "d32": np.ascontiguousarray(d32),
            "bas": _basis100()}


# ------------------------------------------------------------- device kernel
def _split_multiwaits(nc):
    """This walrus build accepts at most ONE sync wait per instruction.
    Tile emits several.  Engine instruction streams execute in order, so
    moving all but one wait onto single-wait NoOps inserted just before
    the instruction preserves semantics exactly."""
    cnt = 0
    for fn in nc.m.functions:
        for blk in fn.blocks:
            out = []
            for inst in blk.instructions:
                si = inst.sync_info
                if si is not None and si.on_wait is not None \
                        and len(si.on_wait) > 1:
                    waits = list(si.on_wait)
                    for w in waits[:-1]:
                        cnt += 1
                        out.append(mybir.InstNoOp(
                            name=f"WSPLIT-{cnt}",
                            engine=inst.engine,
                            ins=[], outs=[],
                            sync_info=mybir.SyncInfo(on_wait=[w],
                                                     on_update=[]),
                        ))
                    inst.sync_info = mybir.SyncInfo(
                        on_wait=[waits[-1]], on_update=list(si.on_update))
                out.append(inst)
            blk.instructions = out
    return cnt


def _build_nc(VP):
    nch = VP // 128
    NL = NXY + 10          # 138 lattice columns
    W16 = VP + NL
    W32 = 3 * nch + NCC + 128
    nc = bass.Bass()
    d16_d = nc.dram_tensor("d16", [17, W16], f16, kind="ExternalInput")
    d32_d = nc.dram_tensor("d32", [128, W32], f32, kind="ExternalInput")
    bas_d = nc.dram_tensor("bas", [NJ * NCC, 512 * NCC], f16,
                           kind="ExternalInput")
    out_d = nc.dram_tensor("out", [MAP_SIZE, MAP_SIZE], f32,
                           kind="ExternalOutput")

    with ExitStack() as ctx:
        tc = ctx.enter_context(tile.TileContext(nc))
        constp = ctx.enter_context(tc.tile_pool(name="const", bufs=1))
        parm = ctx.enter_context(tc.tile_pool(name="parm", bufs=1))
        work = ctx.enter_context(tc.tile_pool(name="work", bufs=6))
        psA = ctx.enter_context(
            tc.tile_pool(name="psA", bufs=1, space=bass.MemorySpace.PSUM))

        # Warmups first (top scheduler priority): ACT exp-table load + PE
        # HAM burst run during the sem-init + input-DMA window.
        scr = constp.tile([1, 1], f32, tag="scr", name="scr")
        nc.vector.memset(scr[:], 0.0)
        nc.scalar.activation(scr[:], scr[:], AF.Exp, bias=0.0, scale=1.0)
        wrm = constp.tile([128, 512], f16, tag="wrm", name="wrm")
        nc.vector.memset(wrm[:], 0.0)
        wps = psA.tile([128, 512], f32, tag="wps", name="wps")
        for _ in range(12):
            nc.tensor.matmul(wps[:], wrm[:, 0:128], wrm[:],
                             start=True, stop=True, skip_group_check=True)

        bas_t = constp.tile([NJ * NCC, 512 * NCC], f16, tag="bas",
                            name="bas")
        nc.gpsimd.dma_start(bas_t[:], bas_d[:])
        dp16 = constp.tile([17, W16], f16, tag="dp16", name="dp16")
        nc.sync.dma_start(dp16[:], d16_d[:])
        dp32 = constp.tile([128, W32], f32, tag="dp32", name="dp32")
        nc.scalar.dma_start(dp32[:], d32_d[:])
        vt_t = dp16[:, 0:VP]
        rhs_t = dp16[:, VP:VP + NL]
        e3_t = dp32[:, 0:3 * nch]
        lg_t = dp32[:, 3 * nch:3 * nch + NCC]
        eye_t = dp32[:, 3 * nch + NCC:W32]

        ones16 = constp.tile([1, 128], f16, tag="ones16", name="ones16")
        nc.vector.memset(ones16[:], 1.0)
        konst = constp.tile([128, 2 * NCC], f32, tag="konst", name="konst")
        nc.vector.memset(konst[:, 0:NCC], -128.0)
        nc.vector.memset(konst[:, NCC:2 * NCC], -127.0)
        ctile = constp.tile([128, NJ * NCC], f32, tag="ctile", name="ctile")
        cj = ctile[:].rearrange("p (c j) -> p c j", j=NJ)
        nc.vector.memset(cj[:, :, 0], 1.0)
        nc.vector.memset(cj[:, :, 1], 1.0)

        # sigmoid(logits): independent of phase 1, runs early.
        en = parm.tile([128, NCC], f32, tag="en", name="en")
        nc.scalar.activation(en[:], lg_t, AF.Exp, bias=0.0, scale=-1.0)
        nc.vector.tensor_scalar_add(en[:], en[:], 1.0)
        pb = parm.tile([128, NCC], f32, tag="pb", name="pb")
        nc.vector.reciprocal(pb[:], en[:])

        # ---------------- phase 1: factorized soft match ----------------
        bsb = parm.tile([128, 3 * NCC], f32, tag="bsb", name="bsb")
        with tc.tile_pool(name="psB", bufs=1,
                          space=bass.MemorySpace.PSUM) as psB:
            B_ps = psB.tile([128, 3 * NCC], f32, tag="B", name="B")
            with tc.tile_pool(name="psW", bufs=3,
                              space=bass.MemorySpace.PSUM) as psW:
                for k in range(nch):
                    ct = psW.tile([128, NL], f32, tag="cross", name="cross")
                    nc.tensor.matmul(ct[:], vt_t[:, k * 128:(k + 1) * 128],
                                     rhs_t, start=True, stop=True)
                    wx = work.tile([128, NL], f32, tag="wx", name="wx")
                    nc.scalar.activation(wx[:], ct[:], AF.Exp,
                                         bias=0.0, scale=EXP_SCALE)
                    wze = work.tile([128, 3 * NCC], f32, tag="wze",
                                    name="wze")
                    e3b = e3_t[:, 3 * k:3 * k + 3] \
                        .rearrange("p (one f) -> p one f", one=1) \
                        .broadcast_to([128, NCC, 3])
                    wzb = wx[:, NXY:NL] \
                        .rearrange("p (k one) -> p k one", one=1) \
                        .broadcast_to([128, NCC, 3])
                    nc.vector.tensor_tensor(
                        wze[:].rearrange("p (k f) -> p k f", f=3),
                        e3b, wzb, ALU.mult)
                    nc.tensor.matmul(B_ps[:], wx[:, 0:NXY], wze[:],
                                     start=(k == 0), stop=(k == nch - 1))
            nc.vector.tensor_copy(bsb[:], B_ps[:])
        bs3 = bsb[:].rearrange("p (k f) -> p k f", f=3)

        def pt(tag, w=NCC):
            return parm.tile([128, w], f32, tag=tag, name=tag)

        # ---------------- per-contact params ----------------
        t0 = pt("t0")
        nc.vector.tensor_scalar_add(t0[:], bs3[:, :, 2], 1e-8)
        rws = pt("rws"); nc.vector.reciprocal(rws[:], t0[:])
        pol = pt("pol")
        nc.vector.tensor_mul(pol[:], bs3[:, :, 0], rws[:])
        ecc = pt("ecc")
        nc.vector.tensor_mul(ecc[:], bs3[:, :, 1], rws[:])

        # t20 = [t | |t| - pi/2], t = pol*rad - pi;  poly gives
        # [sin(t) | -cos(t)]  (factored-root form, 6 fused DVE ops).
        t20 = pt("t20", 2 * NCC)
        nc.vector.tensor_scalar(t20[:, 0:NCC], pol[:], _DEG2RAD, -PI,
                                ALU.mult, ALU.add)
        nc.vector.scalar_tensor_tensor(t20[:, NCC:2 * NCC], t20[:, 0:NCC],
                                       -1.0, t20[:, 0:NCC],
                                       ALU.mult, ALU.max)
        nc.vector.tensor_scalar_add(t20[:, NCC:2 * NCC],
                                    t20[:, NCC:2 * NCC], -PI / 2.0)
        px = pt("px", 2 * NCC)
        nc.vector.tensor_mul(px[:], t20[:], t20[:])
        pu = pt("pu", 2 * NCC)
        nc.vector.tensor_scalar_mul(pu[:], t20[:], C4)
        pa = pt("pa", 2 * NCC)
        nc.vector.scalar_tensor_tensor(pa[:], px[:], -R1, pu[:],
                                       ALU.add, ALU.mult)
        pb2 = pt("pb2", 2 * NCC)
        nc.vector.scalar_tensor_tensor(pb2[:], px[:], -R2, pa[:],
                                       ALU.add, ALU.mult)
        pq = pt("pq", 2 * NCC)
        nc.vector.scalar_tensor_tensor(pq[:], px[:], QB, px[:],
                                       ALU.add, ALU.mult)
        sc20 = pt("sc20", 2 * NCC)
        nc.vector.scalar_tensor_tensor(sc20[:], pq[:], QC, pb2[:],
                                       ALU.add, ALU.mult)

        # t12 = ecc * [sn | -cs];  nxy = t12*SE + [-128 | -127]
        t12 = pt("t12", 2 * NCC)
        eb = ecc[:].rearrange("p (one k) -> p one k", one=1) \
            .broadcast_to([128, 2, NCC])
        nc.vector.tensor_tensor(t12[:].rearrange("p (two k) -> p two k",
                                                 two=2),
                                sc20[:].rearrange("p (two k) -> p two k",
                                                  two=2),
                                eb, ALU.mult)
        nxy = pt("nxy", 2 * NCC)
        nc.vector.scalar_tensor_tensor(nxy[:], t12[:], SE, konst[:],
                                       ALU.mult, ALU.add)

        # 3-level hi/lo center split + squared-head coeffs into the
        # [128, 100] coeff matrix (j = [1,1,syh,syl,syl2,sysh,
        # sxh,sxl,sxl2,sxsh]; halves of nxy are [x | y]).
        # DVE does the center splits; GpSimd does the square chain.
        hi16 = parm.tile([128, 2 * NCC], f16, tag="hi16", name="hi16")
        nc.vector.tensor_copy(hi16[:], nxy[:])
        nc.vector.tensor_copy(cj[:, :, 2], hi16[:, NCC:2 * NCC])
        nc.vector.tensor_copy(cj[:, :, 6], hi16[:, 0:NCC])
        lo1 = pt("lo1", 2 * NCC)
        nc.vector.tensor_sub(lo1[:], nxy[:], hi16[:])
        lo16 = parm.tile([128, 2 * NCC], f16, tag="lo16", name="lo16")
        nc.vector.tensor_copy(lo16[:], lo1[:])
        nc.vector.tensor_copy(cj[:, :, 3], lo16[:, NCC:2 * NCC])
        nc.vector.tensor_copy(cj[:, :, 7], lo16[:, 0:NCC])
        nc.vector.tensor_sub(cj[:, :, 4], lo1[:, NCC:2 * NCC],
                             lo16[:, NCC:2 * NCC])
        nc.vector.tensor_sub(cj[:, :, 8], lo1[:, 0:NCC], lo16[:, 0:NCC])
        sqs = pt("sqs", 2 * NCC)
        nc.gpsimd.tensor_mul(sqs[:], nxy[:], nxy[:])
        sqh16 = parm.tile([128, 2 * NCC], f16, tag="sqh16", name="sqh16")
        nc.gpsimd.tensor_copy(sqh16[:], sqs[:])
        nc.gpsimd.tensor_copy(cj[:, :, 5], sqh16[:, NCC:2 * NCC])
        nc.gpsimd.tensor_copy(cj[:, :, 9], sqh16[:, 0:NCC])
        sql = pt("sql", 2 * NCC)
        nc.gpsimd.tensor_sub(sql[:], sqs[:], sqh16[:])
        s2s = pt("s2s")
        nc.gpsimd.tensor_add(s2s[:], sql[:, 0:NCC], sql[:, NCC:2 * NCC])
        wfac = pt("wfac")
        nc.scalar.activation(wfac[:], s2s[:], AF.Exp, bias=0.0,
                             scale=-1.0 / (1.0 + 1e-8))
        val = pt("val")
        nc.vector.tensor_scalar_min(val[:], bs3[:, :, 2], 1.0)
        wc = pt("wc"); nc.vector.tensor_mul(wc[:], pb[:], val[:])
        wc2 = pt("wc2"); nc.vector.tensor_mul(wc2[:], wc[:], wfac[:])

        lhsT = parm.tile([NJ * NCC, 128], f16, tag="lhsT", name="lhsT")
        with tc.tile_pool(name="psT", bufs=1,
                          space=bass.MemorySpace.PSUM) as psT:
            coefT = psT.tile([NJ * NCC, 128], f32, tag="coefT",
                             name="coefT")
            nc.tensor.transpose(coefT[:], ctile[:], eye_t)
            nc.vector.tensor_copy(lhsT[:], coefT[:])

        # HAM insurance: a DENSE burst of full-width fp16 matmuls that
        # depends on bsb (so the scheduler cannot hoist it before phase
        # 1) fills the params window and keeps the clock-gate epochs
        # seeing a busy PE, so phase 2 runs at the fast clock.
        wrmB = parm.tile([128, 512], f16, tag="wrmB", name="wrmB")
        nc.vector.memset(wrmB[:], 0.0)
        nc.vector.tensor_copy(wrmB[:, 0:3 * NCC], bsb[:])
        for _ in range(9):
            nc.tensor.matmul(wps[:], wrmB[:, 0:128], wrmB[:],
                             start=True, stop=True, skip_group_check=True)

        # ---------------- phase 2: separable splat (chunk pairs) --------
        psM = ctx.enter_context(
            tc.tile_pool(name="psM", bufs=1, space=bass.MemorySpace.PSUM))
        psY = ctx.enter_context(
            tc.tile_pool(name="psY", bufs=2, space=bass.MemorySpace.PSUM))
        mp = psM.tile([128, 2 * MAP_SIZE], f32, tag="map", name="map")
        SC_E = -1.0 / (1.0 + 1e-8)

        def consume(p, yp):
            xy8 = work.tile([128, 4 * MAP_SIZE], f16, tag="xy8", name="xy8")
            nc.scalar.activation(xy8[:], yp[:], AF.Exp, bias=0.0,
                                 scale=SC_E)
            for h in (0, 1):
                c = 2 * p + h
                o = 2 * MAP_SIZE * h
                yy = work.tile([128, MAP_SIZE], f16, tag="yy", name="yy")
                nc.vector.tensor_scalar_mul(yy[:], xy8[:, o:o + MAP_SIZE],
                                            wc2[:, c:c + 1])
                xx = xy8[:, o + MAP_SIZE:o + 2 * MAP_SIZE]
                nc.tensor.matmul(mp[:, 0:MAP_SIZE], yy[:, 0:128], xx,
                                 start=(c == 0), stop=(c == NCC - 1))
                nc.tensor.matmul(mp[:, MAP_SIZE:2 * MAP_SIZE],
                                 yy[:, 128:256], xx,
                                 start=(c == 0), stop=(c == NCC - 1))

        prev = None
        for p in range(NCC // 2):
            yp = psY.tile([128, 4 * MAP_SIZE], f32, tag="yp", name="yp")
            for h in (0, 1):
                c = 2 * p + h
                nc.tensor.matmul(
                    yp[:, 512 * h:512 * (h + 1)], lhsT[:],
                    bas_t[:, 512 * c:512 * (c + 1)],
                    start=True, stop=True)
            if prev is not None:
                consume(*prev)
            prev = (p, yp)
        consume(*prev)

        # ---------------- normalize + store ----------------
        mx = parm.tile([128, 1], f32, tag="mx", name="mx")
        nc.vector.reduce_max(mx[:], mp[:], axis=mybir.AxisListType.X)
        with tc.tile_pool(name="psG", bufs=1,
                          space=bass.MemorySpace.PSUM) as psG:
            mt = psG.tile([1, 128], f32, tag="mt", name="mt")
            nc.tensor.transpose(mt[:], mx[:], eye_t)
            gm = parm.tile([1, 1], f32, tag="gm", name="gm")
            nc.vector.reduce_max(gm[:], mt[:], axis=mybir.AxisListType.X)
            nc.vector.tensor_scalar_add(gm[:], gm[:], 1e-8)
            gi16 = parm.tile([1, 1], f16, tag="gi16", name="gi16")
            with nc.allow_low_precision(reason="norm factor, 5e-4 is fine"):
                nc.vector.reciprocal(gi16[:], gm[:])
            gb = psG.tile([128, 1], f32, tag="gb", name="gb")
            nc.tensor.matmul(gb[:], ones16[:], gi16[:],
                             start=True, stop=True)
            gs = parm.tile([128, 1], f32, tag="gs", name="gs")
            nc.vector.tensor_copy(gs[:], gb[:])

        o0 = work.tile([128, MAP_SIZE], f32, tag="o0", name="o0")
        nc.vector.tensor_scalar_mul(o0[:], mp[:, 0:MAP_SIZE], gs[:])
        o1 = work.tile([128, MAP_SIZE], f32, tag="o1", name="o1")
        nc.scalar.activation(o1[:], mp[:, MAP_SIZE:2 * MAP_SIZE],
                             AF.Copy, scale=gs[:])
        nc.sync.dma_start(out_d[0:128, :], o0[:])
        nc.scalar.dma_start(out_d[128:256, :], o1[:])
    return nc


# ----------------------------------------------------------------- entry
def _run(inputs, trace=False):
    params = np.asarray(inputs["params"], np.float32)
    logits = np.asarray(inputs["electrode_logits"], np.float32)
    v1_pos = np.asarray(inputs["v1_pos"], np.float32)
    v1_prf = np.asarray(inputs["v1_prf"], np.float32)
    start_loc = np.asarray(inputs["start_loc"], np.float32)
    surf_dist_lut = np.asarray(inputs["surf_dist_lut"], np.float32)
    alpha_grid = np.asarray(inputs["alpha_grid"], np.float32)
    beta_grid = np.asarray(inputs["beta_grid"], np.float32)

    gc, R, direction, shank = _host_geometry(
        params, start_loc, surf_dist_lut, alpha_grid, beta_grid)
    keeps = [_voxel_keep(v1_pos, gc[b], R[b, :, 2], shank[b] / 2.0)
             for b in range(B)]
    nkeep = max(int(k.sum()) for k in keeps)
    VP = max(128, ((nkeep + 127) // 128) * 128)

    in_maps = []
    for b in range(B):
        k = keeps[b]
        in_maps.append(_prep_core(gc[b], R[b], shank[b], logits[b],
                                  v1_pos[k], v1_prf[k], VP))
    nc = _build_nc(VP)
    _split_multiwaits(nc)
    res = run_bass_kernel_spmd(nc, in_maps, list(range(B)), trace=trace)
    out = np.stack([res.results[i]["out"] for i in range(B)])
    return out[:, None, :, :].astype(np.float32), res


def kernel(**inputs) -> np.ndarray:
    out, _ = _run(inputs, trace=False)
    return out
